# revision 32
# baseline (speedup 1.0000x reference)
"""Trainium2 Bass kernel for the DeepSets-style segment_reduce network.

Network (per sample, B=2048, M=128 elements):
  phi: 3 -> 120 -> 100 -> 80 MLP (all ReLU), applied per element
  pooled = sum over the 128 elements                      [B, 80]
  rho:  80 -> 60 -> 60 -> 40 (ReLU, ReLU, linear)
  q:    concat(rho_out, static) 43 -> 200 -> 100 -> 3, softmax

Mapping: data-parallel over 8 NeuronCores (256 samples each).
v2 design (from trace analysis of the v1 kernel):
  - L1 runs as fp8e4 DoubleRow matmuls (0.5 cycles/col) packed 4-way across
    PE row-groups; x is host-packed into the [2-part, 2-subtile] DR layout.
  - L2/L3 stay fp16 (1 cycle/col), stationary weights, 512-col moving ops.
  - PSUM drain (the real wall: ~98k columns through ACT+DVE at 1/cycle):
    L1/L2 tiles are whole-tile relu drains assigned to ACT or DVE by a
    load-balance heuristic; L3 drains use *tile-pair folding*: element
    order pairs tile A (m<64) with tile B (m>=64) column-for-column, so
    ACT relus tile A whole, and one whole-tile DVE STT does
    relu(tileB)+tileA — the first pooling level fused into the drain.
  - Pool (GpSimd) has no PSUM port; it takes tree level-1 (fp16 SBUF adds).
  - Tails run per 64-sample quarter, their serial matmul->drain links
    interleaved between phi tile emissions so the PE never idles on the
    chain; only the last quarter's tree+tail chain is exposed.
  - x DMA is chunked across both HWDGE rings (sync+scalar) so the first
    L1 matmul is not gated by a 5us serial DMA; the PE warmup bridges
    directly into L1 to keep the HAM/p-state ramp alive.
"""

import sys
import numpy as np

sys.path.insert(0, '/opt/trn_rl_repo')

B, M, D = 2048, 128, 3
N_CORES = 8
BC = B // N_CORES            # samples per core (256)
EC = BC * M                  # elements per core (32768)
HALF = EC // 2               # elements per half (16384), 128 samples
SAMP_HALF = BC // 2          # samples per half (128)
QS = 64                      # tail quarter size (samples)
PT = 1024                    # cols per PSUM tile
TPH = HALF // PT             # psum tiles per half (16)
NPAIR = TPH // 2             # L3 tile pairs per half (8)
L1B = 512                    # L1 matmul block width (cols)

F1, F2, F3 = 120, 100, 80    # phi widths
R1, R2, R3 = 60, 60, 40      # rho widths
Q1, Q2, Q3 = 200, 100, 3     # q widths
XQ = R3 + 3 + 1              # q input rows: rho_out + static + ones (44)

# fp16 weight blob column layout: name -> (rows, cols, col_offset)
# phi weights are padded to K=128/M=128 so FWL (fast weight load) triggers;
# a unit weight on the previous layer's ones-row makes each layer's ones-row
# self-compute in PSUM (h1[127]=1, h2[127]=1), so no ones-row DMAs needed.
_BLOB = {}
_off = 0
for _name, _r, _c in [("w1a", 4, 128), ("w2a", 128, 128), ("w3a", 128, 128),
                      ("r1a", F3 + 1, R1), ("r2a", R1 + 1, R2), ("r3a", R2 + 1, R3),
                      ("q1aw", XQ, 128), ("q1bw", XQ, Q1 - 128),
                      ("q2aw", 128, Q2), ("q2bw", Q1 - 128 + 1, Q2),
                      ("q3aw", Q2 + 1, Q3), ("statt", 3, BC)]:
    _BLOB[_name] = (_r, _c, _off)
    _off += _c
BLOBW = _off

_compiled = {}


def _build():
    import concourse.bacc as bacc
    import concourse.mybir as mybir
    from concourse import tile

    f32 = mybir.dt.float32
    fp16 = mybir.dt.float16
    Relu = mybir.ActivationFunctionType.Relu
    Exp = mybir.ActivationFunctionType.Exp
    Alu = mybir.AluOpType

    nc = bacc.Bacc("TRN2", target_bir_lowering=False, debug=False,
                   enable_asserts=False, num_devices=N_CORES)

    # x: [half, group, row(x0,x1,x2,ones), cols] fp16, 256-col blocks RR
    xin = nc.dram_tensor("xin", [2, 4, 4, HALF // 4], fp16,
                         kind="ExternalInput").ap()
    blob = nc.dram_tensor("blob", [128, BLOBW], fp16, kind="ExternalInput").ap()
    onesr = nc.dram_tensor("onesr", [1, HALF], fp16, kind="ExternalInput").ap()
    eye3 = nc.dram_tensor("eye3", [3, 3], f32, kind="ExternalInput").ap()
    out = nc.dram_tensor("out", [BC, 3], f32, kind="ExternalOutput").ap()

    # drain engine load-balance state (emit-time estimates, ns)
    bal = {"act": 0.0, "dve": 0.0}

    with tile.TileContext(nc) as tc:
        with tc.tile_pool(name="cst", bufs=1) as cst, \
             tc.tile_pool(name="xp", bufs=2) as xp, \
             tc.tile_pool(name="scr", bufs=1) as scr, \
             tc.tile_pool(name="ps", bufs=3, space="PSUM") as ps, \
             tc.tile_pool(name="pst", bufs=2, space="PSUM") as pst:

            # ---- input DMAs. HWDGE transfers run FIFO per ring, so order by
            # need: w1a first (gates L1), then x half-0 split across both
            # rings, then the rest of the weights.
            blob_sb = cst.tile([128, BLOBW], fp16)
            nc.scalar.dma_start(out=blob_sb[:, 0:128], in_=blob[:, 0:128])
            x_sb0 = xp.tile([128, HALF // 4], fp16, name="x_sb0", tag="x", bufs=2)
            for j in range(4):
                eng = nc.sync if j < 2 else nc.scalar
                eng.dma_start(out=x_sb0[32 * j:32 * j + 4, :], in_=xin[0, j])
            PHI_W_COLS = 384            # w1a+w2a+w3a
            nc.scalar.dma_start(out=blob_sb[:, 128:PHI_W_COLS],
                                in_=blob[:, 128:PHI_W_COLS])
            nc.sync.dma_start(out=blob_sb[:, PHI_W_COLS:],
                              in_=blob[:, PHI_W_COLS:])

            def wslice(name):
                r, c, o = _BLOB[name]
                return blob_sb[0:r, o:o + c]

            w2s, w3s = wslice("w2a"), wslice("w3a")
            r1s, r2s, r3s = wslice("r1a"), wslice("r2a"), wslice("r3a")
            q1as, q1bs = wslice("q1aw"), wslice("q1bw")
            q2as, q2bs = wslice("q2aw"), wslice("q2bw")
            q3as, statt = wslice("q3aw"), wslice("statt")

            # PE warm-up source
            wtiny = cst.tile([128, 512], fp16)
            nc.vector.memset(wtiny[:, :], 0.0)

            # persistent activation planes (ones rows self-compute via the
            # padded weight columns; no ones DMAs needed)
            h1s, h2s, s_hs = [], [], []
            for h in range(2):
                h1s.append(cst.tile([128, HALF], fp16, name=f"h1_{h}"))
                h2s.append(cst.tile([128, HALF], fp16, name=f"h2_{h}"))
                s_hs.append(cst.tile([F3, HALF // 2], fp16, name=f"s_{h}"))

            pooled = cst.tile([F3 + 1, BC], fp16)      # ones row at 80
            nc.gpsimd.dma_start(out=pooled[F3:F3 + 1, :], in_=onesr[:, 0:BC])
            eye3s = cst.tile([3, 3], f32)
            nc.gpsimd.dma_start(out=eye3s[:, :], in_=eye3)
            eye1 = cst.tile([1, 1], f32)
            nc.vector.memset(eye1[:, :], 1.0)
            ones3 = cst.tile([3, 1], f32)
            nc.vector.memset(ones3[:, :], 1.0)

            # persistent tail planes [*, BC] (ones rows written once)
            xqp = cst.tile([XQ, BC], fp16)
            _r, _c, _o = _BLOB["statt"]
            nc.gpsimd.dma_start(out=xqp[R3:R3 + 3, :], in_=blob[0:3, _o:_o + _c])
            nc.gpsimd.dma_start(out=xqp[XQ - 1:XQ, :], in_=onesr[:, 0:BC])
            hr1p = cst.tile([R1 + 1, BC], fp16)
            nc.gpsimd.dma_start(out=hr1p[R1:R1 + 1, :], in_=onesr[:, 0:BC])
            hr2p = cst.tile([R2 + 1, BC], fp16)
            nc.gpsimd.dma_start(out=hr2p[R2:R2 + 1, :], in_=onesr[:, 0:BC])
            hq1ap = cst.tile([128, BC], fp16)
            hq1bp = cst.tile([Q1 - 128 + 1, BC], fp16)
            nc.gpsimd.dma_start(out=hq1bp[Q1 - 128:Q1 - 128 + 1, :],
                                in_=onesr[:, 0:BC])
            hq2p = cst.tile([Q2 + 1, BC], fp16)
            nc.gpsimd.dma_start(out=hq2p[Q2:Q2 + 1, :], in_=onesr[:, 0:BC])
            e_sbp = cst.tile([3, BC], f32)

            # warm-up matmuls: back-to-back, bridging into L1 (ramp the PE)
            pw = ps.tile([128, 512], f32, name="pw", tag="hp")
            for i in range(8):
                nc.tensor.matmul(pw[:, :], wtiny[:, 0:128], wtiny[:, :],
                                 start=(i == 0), stop=(i == 7))
            wsink = cst.tile([32, 2], fp16)
            nc.vector.tensor_scalar_max(wsink[:, :], pw[0:32, 0:2], 0.0)

            # ---- drain helper: whole-tile relu drain on ACT or DVE by balance
            def drain(dst, src, ncols):
                est_a = 0.85 * ncols + 220.0
                est_d = 1.04 * ncols + 190.0
                if bal["act"] + est_a <= bal["dve"] + est_d:
                    bal["act"] += est_a
                    nc.scalar.activation(dst, src, Relu)
                else:
                    bal["dve"] += est_d
                    nc.vector.tensor_scalar_max(dst, src, 0.0)

            # ---- phi layer sweeps ----
            def phi_l1(h, x_sb, slot=None):
                # L1 blocks of L1B cols round-robin over PE row-groups
                bpt = PT // L1B
                for t in range(TPH):
                    if slot is not None:
                        slot(1)
                    p1 = ps.tile([128, PT], f32, name="p1", tag="hp")
                    for cc in range(bpt):
                        b = bpt * t + cc
                        j = b % 4
                        bl = b // 4
                        nc.tensor.matmul(
                            p1[:, cc * L1B:(cc + 1) * L1B],
                            blob_sb[32 * j:32 * j + 4, 0:128],
                            x_sb[32 * j:32 * j + 4, bl * L1B:(bl + 1) * L1B],
                            start=True, stop=True, tile_position=(32 * j, 0))
                    drain(h1s[h][:, t * PT:(t + 1) * PT], p1[:, :], PT)

            def phi_l2(h, slot=None):
                for t in range(TPH):
                    if slot is not None:
                        slot(1)
                    p2 = ps.tile([128, PT], f32, name="p2", tag="hp")
                    for cc in range(2):
                        c0 = t * PT + cc * 512
                        nc.tensor.matmul(p2[:, cc * 512:(cc + 1) * 512],
                                         w2s, h1s[h][:, c0:c0 + 512],
                                         start=True, stop=True)
                    drain(h2s[h][:, t * PT:(t + 1) * PT], p2[:, :], PT)

            def phi_l3_pair(h, p):
                # tile A: m in [0,64); tile B: m in [64,128) of 16 samples.
                # ACT relus A whole-tile into scratch; one DVE STT folds
                # relu(B) + A into s (first pooling level fused into drain).
                pA = ps.tile([128, PT], f32, name="p3a", tag="hp")
                pB = ps.tile([128, PT], f32, name="p3b", tag="hp")
                for cc in range(2):
                    cA = (2 * p) * PT + cc * 512
                    nc.tensor.matmul(pA[:, cc * 512:(cc + 1) * 512],
                                     w3s, h2s[h][:, cA:cA + 512],
                                     start=True, stop=True)
                for cc in range(2):
                    cB = (2 * p + 1) * PT + cc * 512
                    nc.tensor.matmul(pB[:, cc * 512:(cc + 1) * 512],
                                     w3s, h2s[h][:, cB:cB + 512],
                                     start=True, stop=True)
                h3ra = scr.tile([F3, PT], fp16, name="h3ra", tag="h3ra", bufs=2)
                nc.scalar.activation(h3ra[:, :], pA[0:F3, :], Relu)
                bal["act"] += 0.85 * PT + 220.0
                nc.vector.scalar_tensor_tensor(
                    s_hs[h][:, p * PT:(p + 1) * PT],
                    pB[0:F3, :], 0.0, h3ra[:, :],
                    op0=Alu.max, op1=Alu.add)
                bal["dve"] += 1.04 * PT + 190.0

            # ---- pooling trees (from 64 per sample) per 64-sample quarter
            def trees_q(q, last=False):
                h = q // 2
                s_sl = s_hs[h][:, (q % 2) * 4096:(q % 2) * 4096 + 4096]
                a3 = s_sl.rearrange("p (g m) -> p g m", m=64)
                t1 = scr.tile([F3, 2048], fp16, name=f"t1_{q}", tag="t1", bufs=2)
                o3 = t1[:, :].rearrange("p (g m) -> p g m", m=32)
                if last:
                    # DVE level-1 (2x fp16) for low exposed latency
                    nc.vector.tensor_tensor(out=o3, in0=a3[:, :, 0:32],
                                            in1=a3[:, :, 32:64], op=Alu.add)
                    bal["dve"] += 1250.0
                else:
                    nc.gpsimd.tensor_tensor(out=o3, in0=a3[:, :, 0:32],
                                            in1=a3[:, :, 32:64], op=Alu.add)
                w = 32
                srt = t1[:, :]
                lv = 0
                while w > 1:
                    w //= 2
                    if w > 1:
                        nxt = scr.tile([F3, 64 * w], fp16, name=f"tl{lv}",
                                       tag=f"tl{lv}", bufs=2)[:, :]
                    else:
                        nxt = pooled[0:F3, q * QS:(q + 1) * QS]
                    a = srt.rearrange("p (g m) -> p g m", m=2 * w)
                    nc.vector.tensor_tensor(
                        out=nxt.rearrange("p (g m) -> p g m", m=w) if w > 1 else nxt,
                        in0=a[:, :, 0:w], in1=a[:, :, w:2 * w], op=Alu.add)
                    srt = nxt
                    lv += 1
                bal["dve"] += 1850.0

            # ---- tail for one 64-sample quarter, emitted as a list of
            # closures so links can be interleaved between phi tiles.
            def tail_links(q):
                sl = slice(q * QS, (q + 1) * QS)
                links = []

                def l_r1():
                    pr1 = pst.tile([R1, QS], f32, name=f"pr1_{q}", tag="tail")
                    nc.tensor.matmul(pr1[:, :], r1s, pooled[:, sl],
                                     start=True, stop=True)
                    nc.scalar.activation(hr1p[0:R1, sl], pr1[:, :], Relu)

                def l_r2():
                    pr2 = pst.tile([R2, QS], f32, name=f"pr2_{q}", tag="tail")
                    nc.tensor.matmul(pr2[:, :], r2s, hr1p[:, sl],
                                     start=True, stop=True)
                    nc.scalar.activation(hr2p[0:R2, sl], pr2[:, :], Relu)

                def l_r3():
                    pr3 = pst.tile([R3, QS], f32, name=f"pr3_{q}", tag="tail")
                    nc.tensor.matmul(pr3[:, :], r3s, hr2p[:, sl],
                                     start=True, stop=True)
                    nc.scalar.copy(xqp[0:R3, sl], pr3[:, :])

                def l_q1():
                    pq1a = pst.tile([128, QS], f32, name=f"pq1a_{q}", tag="tail")
                    pq1b = pst.tile([Q1 - 128, QS], f32, name=f"pq1b_{q}", tag="tail")
                    nc.tensor.matmul(pq1a[:, :], q1as, xqp[:, sl],
                                     start=True, stop=True)
                    nc.tensor.matmul(pq1b[:, :], q1bs, xqp[:, sl],
                                     start=True, stop=True)
                    nc.scalar.activation(hq1ap[:, sl], pq1a[:, :], Relu)
                    nc.vector.tensor_scalar_max(hq1bp[0:Q1 - 128, sl],
                                                pq1b[:, :], 0.0)

                def l_q2():
                    pq2 = pst.tile([Q2, QS], f32, name=f"pq2_{q}", tag="tail")
                    nc.tensor.matmul(pq2[:, :], q2as, hq1ap[:, sl],
                                     start=True, stop=False)
                    nc.tensor.matmul(pq2[:, :], q2bs, hq1bp[:, sl],
                                     start=False, stop=True)
                    nc.scalar.activation(hq2p[0:Q2, sl], pq2[:, :], Relu)

                def l_q3():
                    pq3 = pst.tile([Q3, QS], f32, name=f"pq3_{q}", tag="tail")
                    nc.tensor.matmul(pq3[:, :], q3as, hq2p[:, sl],
                                     start=True, stop=True)
                    nc.scalar.activation(e_sbp[:, sl], pq3[:, :], Exp)

                rec = cst.tile([1, QS], f32, name=f"rec{q}")

                def l_sm1():
                    ssum = pst.tile([1, QS], f32, name=f"ssum{q}", tag="tail")
                    nc.tensor.matmul(ssum[:, :], ones3[:, :], e_sbp[:, sl],
                                     start=True, stop=True)
                    nc.vector.reciprocal(rec[:, :], ssum[:, :])

                def l_sm2():
                    eT = pst.tile([QS, 3], f32, name=f"eT{q}", tag="tail")
                    nc.tensor.transpose(eT[:, :], e_sbp[:, sl], eye3s[:, :])
                    rT = pst.tile([QS, 1], f32, name=f"rT{q}", tag="tail")
                    nc.tensor.transpose(rT[:, :], rec[:, :], eye1[:, :])
                    rTs = cst.tile([QS, 1], f32, name=f"rTs{q}")
                    nc.vector.tensor_copy(rTs[:, :], rT[:, :])
                    o_sb = cst.tile([QS, 3], f32, name=f"o_sb{q}")
                    nc.vector.tensor_scalar_mul(o_sb[:, :], eT[:, :], rTs[:, :])
                    nc.sync.dma_start(out=out[q * QS:(q + 1) * QS, :],
                                      in_=o_sb[:, :])

                links += [l_r1, l_r2, l_r3, l_q1, l_q2, l_q3, l_sm1, l_sm2]
                bal["act"] += 1500.0
                bal["dve"] += 700.0
                return links

            # ---- emission schedule ----
            pending = []               # tail links awaiting interleave slots

            def emit_pending(n=1):
                for _ in range(n):
                    if pending:
                        pending.pop(0)()

            phi_l1(0, x_sb0)
            # x half 1 DMAs early (sync ring is idle; transfers overlap L2/L3)
            x_sb1 = xp.tile([128, HALF // 4], fp16, name="x_sb1", tag="x", bufs=2)
            for j in range(4):
                nc.sync.dma_start(out=x_sb1[32 * j:32 * j + 4, :], in_=xin[1, j])
            phi_l2(0)
            for p in range(NPAIR):
                phi_l3_pair(0, p)
                if p == 3:
                    trees_q(0)
                    pending += tail_links(0)
                emit_pending(1)
            trees_q(1)
            pending += tail_links(1)
            # half 1: interleave remaining tail links 1-per-slot into the
            # L1/L2/L3 sweeps (a tail link between phi tiles hides its
            # matmul->drain round trip under the next tile's matmuls)
            phi_l1(1, x_sb1, slot=emit_pending)
            phi_l2(1, slot=emit_pending)
            for p in range(NPAIR):
                phi_l3_pair(1, p)
                if p == 3:
                    trees_q(2)
                    pending += tail_links(2)
                emit_pending(1)
            trees_q(3, last=True)
            pending += tail_links(3)
            emit_pending(len(pending))

    nc.compile()
    return nc


def _prep_inputs(dyn, static, phi_w1, phi_b1, phi_w2, phi_b2, phi_w3, phi_b3,
                 rho_w1, rho_b1, rho_w2, rho_b2, rho_w3, rho_b3,
                 q_w1, q_b1, q_w2, q_b2, q_w3, q_b3):
    """Build per-core input maps (host-side layout transforms)."""
    fp16 = np.float16

    def aug_t(w, b):
        # [out, in] weight + bias -> transposed augmented [in+1, out]
        return np.concatenate([w, b[:, None]], axis=1).T.astype(fp16)

    q1 = aug_t(q_w1, q_b1)               # [44, 200]
    q2 = aug_t(q_w2, q_b2)               # [201, 100]
    # phi weights padded to [*, 128] / [128, 128] for FWL; bias row moves to
    # row 127 (fed by the previous layer's self-computed ones row), and
    # column 127 carries a unit weight from the previous ones row so the
    # ones row propagates (h1[127] = h2[127] = 1).
    w1a = np.zeros((4, 128), dtype=fp16)
    w1a[:, 0:F1] = aug_t(phi_w1, phi_b1)
    w1a[3, 127] = 1.0                    # x ones row -> h1[127] = 1
    w2a = np.zeros((128, 128), dtype=fp16)
    w2a[0:F1, 0:F2] = phi_w2.T.astype(fp16)
    w2a[127, 0:F2] = phi_b2.astype(fp16)
    w2a[127, 127] = 1.0                  # h1 ones row -> h2[127] = 1
    w3a = np.zeros((128, 128), dtype=fp16)
    w3a[0:F2, 0:F3] = phi_w3.T.astype(fp16)
    w3a[127, 0:F3] = phi_b3.astype(fp16)
    parts = dict(
        w1a=w1a, w2a=w2a, w3a=w3a, r1a=aug_t(rho_w1, rho_b1),
        r2a=aug_t(rho_w2, rho_b2), r3a=aug_t(rho_w3, rho_b3),
        q1aw=q1[:, 0:128], q1bw=q1[:, 128:],
        q2aw=q2[0:128, :], q2bw=q2[128:, :], q3aw=aug_t(q_w3, q_b3))

    eye3 = np.eye(3, dtype=np.float32)
    onesr = np.ones((1, HALF), dtype=fp16)

    base_blob = np.zeros((128, BLOBW), dtype=fp16)
    for name, (r, cc, o) in _BLOB.items():
        if name != "statt":
            base_blob[0:r, o:o + cc] = parts[name]
    for j in range(1, 4):   # replicate L1 weights into each PE row-group
        base_blob[32 * j:32 * j + 4, 0:128] = parts["w1a"]

    # element order within a half: tile T (1024 cols), pair p = T//2:
    # tile A (T even): col c -> sample p*16 + c//64, m = c%64
    # tile B (T odd):  col c -> sample p*16 + c//64, m = 64 + c%64
    # Build permutation: halfcol g -> (sample_in_half, m)
    Tl = np.arange(HALF) // PT
    cl = np.arange(HALF) % PT
    pl = Tl // 2
    samp = pl * 16 + cl // 64
    mm = (Tl % 2) * 64 + cl % 64
    elem_of_col = samp * M + mm          # index into half's [2048*M?] no: per-half

    in_maps = []
    for c in range(N_CORES):
        blob = base_blob.copy()
        r, cc, o = _BLOB["statt"]
        blob[0:r, o:o + cc] = static[c * BC:(c + 1) * BC].T.astype(fp16)
        xc = dyn[c * BC:(c + 1) * BC].reshape(EC, D)
        # xin[h, j, row, t*256 + n] = comp(row) of element at tile t, block j
        xin = np.empty((2, 4, 4, HALF // 4), dtype=fp16)
        for hh in range(2):
            xh = xc[hh * HALF:(hh + 1) * HALF]          # [16384, 3]
            xperm = xh[elem_of_col]                      # cols in emission order
            comp = np.concatenate([xperm, np.ones((HALF, 1), np.float32)],
                                  axis=1)                # [HALF, 4] comps
            # blocks of L1B cols RR over groups: block b -> group b%4,
            # group-local slot b//4
            blocks = comp.reshape(HALF // L1B, L1B, 4)
            for j in range(4):
                grp = blocks[j::4]                   # [HALF//L1B//4, L1B, 4]
                xin[hh, j] = grp.transpose(2, 0, 1).reshape(4, HALF // 4)
        in_maps.append(dict(xin=xin, blob=blob, onesr=onesr, eye3=eye3))
    return in_maps


def kernel(**inputs):
    import time
    from concourse.bass_utils import run_bass_kernel_spmd

    if "nc" not in _compiled:
        _compiled["nc"] = _build()
    nc = _compiled["nc"]

    in_maps = _prep_inputs(**inputs)
    last_err = None
    for attempt in range(3):
        try:
            res = run_bass_kernel_spmd(nc, in_maps, core_ids=list(range(N_CORES)))
            break
        except Exception as e:          # transient device errors: back off and retry
            last_err = e
            time.sleep(20 * (attempt + 1))
    else:
        raise last_err
    out = np.concatenate([res.results[c]["out"] for c in range(N_CORES)], axis=0)
    return out.astype(np.float32)


# revision 37
# speedup vs baseline: 1.0123x; 1.0123x over previous
"""Trainium2 Bass kernel for the DeepSets-style segment_reduce network.

Network (per sample, B=2048, M=128 elements):
  phi: 3 -> 120 -> 100 -> 80 MLP (all ReLU), applied per element
  pooled = sum over the 128 elements                      [B, 80]
  rho:  80 -> 60 -> 60 -> 40 (ReLU, ReLU, linear)
  q:    concat(rho_out, static) 43 -> 200 -> 100 -> 3, softmax

Mapping: data-parallel over 8 NeuronCores (256 samples each).
v2 design (from trace analysis of the v1 kernel):
  - L1 runs as fp8e4 DoubleRow matmuls (0.5 cycles/col) packed 4-way across
    PE row-groups; x is host-packed into the [2-part, 2-subtile] DR layout.
  - L2/L3 stay fp16 (1 cycle/col), stationary weights, 512-col moving ops.
  - PSUM drain (the real wall: ~98k columns through ACT+DVE at 1/cycle):
    L1/L2 tiles are whole-tile relu drains assigned to ACT or DVE by a
    load-balance heuristic; L3 drains use *tile-pair folding*: element
    order pairs tile A (m<64) with tile B (m>=64) column-for-column, so
    ACT relus tile A whole, and one whole-tile DVE STT does
    relu(tileB)+tileA — the first pooling level fused into the drain.
  - Pool (GpSimd) has no PSUM port; it takes tree level-1 (fp16 SBUF adds).
  - Tails run per 64-sample quarter, their serial matmul->drain links
    interleaved between phi tile emissions so the PE never idles on the
    chain; only the last quarter's tree+tail chain is exposed.
  - x DMA is chunked across both HWDGE rings (sync+scalar) so the first
    L1 matmul is not gated by a 5us serial DMA; the PE warmup bridges
    directly into L1 to keep the HAM/p-state ramp alive.
"""

import sys
import numpy as np

sys.path.insert(0, '/opt/trn_rl_repo')

B, M, D = 2048, 128, 3
N_CORES = 8
BC = B // N_CORES            # samples per core (256)
EC = BC * M                  # elements per core (32768)
HALF = EC // 2               # elements per half (16384), 128 samples
SAMP_HALF = BC // 2          # samples per half (128)
QS = 64                      # tail quarter size (samples)
PT = 1024                    # cols per PSUM tile
TPH = HALF // PT             # psum tiles per half (16)
NPAIR = TPH // 2             # L3 tile pairs per half (8)
L1B = 512                    # L1 matmul block width (cols)

F1, F2, F3 = 120, 100, 80    # phi widths
R1, R2, R3 = 60, 60, 40      # rho widths
Q1, Q2, Q3 = 200, 100, 3     # q widths
XQ = R3 + 3 + 1              # q input rows: rho_out + static + ones (44)

# fp16 weight blob column layout: name -> (rows, cols, col_offset)
# phi weights are padded to K=128/M=128 so FWL (fast weight load) triggers;
# a unit weight on the previous layer's ones-row makes each layer's ones-row
# self-compute in PSUM (h1[127]=1, h2[127]=1), so no ones-row DMAs needed.
_BLOB = {}
_off = 0
for _name, _r, _c in [("w1a", 4, 128), ("w2a", 128, 128), ("w3a", 128, 128),
                      ("r1a", F3 + 1, R1), ("r2a", R1 + 1, R2), ("r3a", R2 + 1, R3),
                      ("q1aw", XQ, 128), ("q1bw", XQ, Q1 - 128),
                      ("q2aw", 128, Q2), ("q2bw", Q1 - 128 + 1, Q2),
                      ("q3aw", Q2 + 1, Q3), ("statt", 3, BC)]:
    _BLOB[_name] = (_r, _c, _off)
    _off += _c
BLOBW = _off

_compiled = {}


def _build():
    import concourse.bacc as bacc
    import concourse.mybir as mybir
    from concourse import tile

    f32 = mybir.dt.float32
    fp16 = mybir.dt.float16
    Relu = mybir.ActivationFunctionType.Relu
    Exp = mybir.ActivationFunctionType.Exp
    Alu = mybir.AluOpType

    nc = bacc.Bacc("TRN2", target_bir_lowering=False, debug=False,
                   enable_asserts=False, num_devices=N_CORES)

    # x: [half, group, row(x0,x1,x2,ones), cols] fp16, 256-col blocks RR
    xin = nc.dram_tensor("xin", [2, 4, 4, HALF // 4], fp16,
                         kind="ExternalInput").ap()
    blob = nc.dram_tensor("blob", [128, BLOBW], fp16, kind="ExternalInput").ap()
    onesr = nc.dram_tensor("onesr", [1, HALF], fp16, kind="ExternalInput").ap()
    eye3 = nc.dram_tensor("eye3", [3, 3], f32, kind="ExternalInput").ap()
    out = nc.dram_tensor("out", [BC, 3], f32, kind="ExternalOutput").ap()

    # drain engine load-balance state (emit-time estimates, ns)
    bal = {"act": 0.0, "dve": 0.0}

    with tile.TileContext(nc) as tc:
        with tc.tile_pool(name="cst", bufs=1) as cst, \
             tc.tile_pool(name="xp", bufs=2) as xp, \
             tc.tile_pool(name="scr", bufs=1) as scr, \
             tc.tile_pool(name="ps", bufs=3, space="PSUM") as ps, \
             tc.tile_pool(name="pst", bufs=2, space="PSUM") as pst:

            # ---- input DMAs. HWDGE transfers run FIFO per ring, so order by
            # need: w1a first (gates L1), then x half-0 split across both
            # rings, then the rest of the weights.
            blob_sb = cst.tile([128, BLOBW], fp16)
            nc.scalar.dma_start(out=blob_sb[:, 0:128], in_=blob[:, 0:128])
            x_sb0 = xp.tile([128, HALF // 4], fp16, name="x_sb0", tag="x", bufs=2)
            for j in range(4):
                eng = nc.sync if j < 2 else nc.scalar
                eng.dma_start(out=x_sb0[32 * j:32 * j + 4, :], in_=xin[0, j])
            PHI_W_COLS = 384            # w1a+w2a+w3a
            nc.scalar.dma_start(out=blob_sb[:, 128:PHI_W_COLS],
                                in_=blob[:, 128:PHI_W_COLS])
            nc.sync.dma_start(out=blob_sb[:, PHI_W_COLS:],
                              in_=blob[:, PHI_W_COLS:])

            def wslice(name):
                r, c, o = _BLOB[name]
                return blob_sb[0:r, o:o + c]

            w2s, w3s = wslice("w2a"), wslice("w3a")
            r1s, r2s, r3s = wslice("r1a"), wslice("r2a"), wslice("r3a")
            q1as, q1bs = wslice("q1aw"), wslice("q1bw")
            q2as, q2bs = wslice("q2aw"), wslice("q2bw")
            q3as, statt = wslice("q3aw"), wslice("statt")

            # PE warm-up source
            wtiny = cst.tile([128, 512], fp16)
            nc.vector.memset(wtiny[:, :], 0.0)

            # persistent activation planes (ones rows self-compute via the
            # padded weight columns; no ones DMAs needed)
            h1s, h2s, s_hs = [], [], []
            for h in range(2):
                h1s.append(cst.tile([128, HALF], fp16, name=f"h1_{h}"))
                h2s.append(cst.tile([128, HALF], fp16, name=f"h2_{h}"))
                s_hs.append(cst.tile([F3, HALF // 2], fp16, name=f"s_{h}"))

            pooled = cst.tile([F3 + 1, BC], fp16)      # ones row at 80
            nc.gpsimd.dma_start(out=pooled[F3:F3 + 1, :], in_=onesr[:, 0:BC])
            eye3s = cst.tile([3, 3], f32)
            nc.gpsimd.dma_start(out=eye3s[:, :], in_=eye3)
            eye1 = cst.tile([1, 1], f32)
            nc.vector.memset(eye1[:, :], 1.0)
            ones3 = cst.tile([3, 1], f32)
            nc.vector.memset(ones3[:, :], 1.0)

            # persistent tail planes [*, BC] (ones rows written once)
            xqp = cst.tile([XQ, BC], fp16)
            _r, _c, _o = _BLOB["statt"]
            nc.gpsimd.dma_start(out=xqp[R3:R3 + 3, :], in_=blob[0:3, _o:_o + _c])
            nc.gpsimd.dma_start(out=xqp[XQ - 1:XQ, :], in_=onesr[:, 0:BC])
            hr1p = cst.tile([R1 + 1, BC], fp16)
            nc.gpsimd.dma_start(out=hr1p[R1:R1 + 1, :], in_=onesr[:, 0:BC])
            hr2p = cst.tile([R2 + 1, BC], fp16)
            nc.gpsimd.dma_start(out=hr2p[R2:R2 + 1, :], in_=onesr[:, 0:BC])
            hq1ap = cst.tile([128, BC], fp16)
            hq1bp = cst.tile([Q1 - 128 + 1, BC], fp16)
            nc.gpsimd.dma_start(out=hq1bp[Q1 - 128:Q1 - 128 + 1, :],
                                in_=onesr[:, 0:BC])
            hq2p = cst.tile([Q2 + 1, BC], fp16)
            nc.gpsimd.dma_start(out=hq2p[Q2:Q2 + 1, :], in_=onesr[:, 0:BC])
            e_sbp = cst.tile([3, BC], f32)

            # warm-up matmuls: back-to-back, bridging into L1 (ramp the PE)
            pw = ps.tile([128, 512], f32, name="pw", tag="hp")
            for i in range(4):
                nc.tensor.matmul(pw[:, :], wtiny[:, 0:128], wtiny[:, :],
                                 start=(i == 0), stop=(i == 3))
            wsink = cst.tile([32, 2], fp16)
            nc.vector.tensor_scalar_max(wsink[:, :], pw[0:32, 0:2], 0.0)

            # ---- drain helper: whole-tile relu drain on ACT or DVE by balance
            def drain(dst, src, ncols):
                est_a = 0.85 * ncols + 220.0
                est_d = 1.04 * ncols + 190.0
                if bal["act"] + est_a <= bal["dve"] + est_d:
                    bal["act"] += est_a
                    nc.scalar.activation(dst, src, Relu)
                else:
                    bal["dve"] += est_d
                    nc.vector.tensor_scalar_max(dst, src, 0.0)

            # ---- phi layer sweeps ----
            def phi_l1(h, x_sb, slot=None):
                # L1 blocks of L1B cols round-robin over PE row-groups
                bpt = PT // L1B
                for t in range(TPH):
                    if slot is not None:
                        slot(1)
                    p1 = ps.tile([128, PT], f32, name="p1", tag="hp")
                    for cc in range(bpt):
                        b = bpt * t + cc
                        j = b % 4
                        bl = b // 4
                        nc.tensor.matmul(
                            p1[:, cc * L1B:(cc + 1) * L1B],
                            blob_sb[32 * j:32 * j + 4, 0:128],
                            x_sb[32 * j:32 * j + 4, bl * L1B:(bl + 1) * L1B],
                            start=True, stop=True, tile_position=(32 * j, 0))
                    drain(h1s[h][:, t * PT:(t + 1) * PT], p1[:, :], PT)

            def phi_l2(h, slot=None):
                for t in range(TPH):
                    if slot is not None:
                        slot(1)
                    p2 = ps.tile([128, PT], f32, name="p2", tag="hp")
                    for cc in range(2):
                        c0 = t * PT + cc * 512
                        nc.tensor.matmul(p2[:, cc * 512:(cc + 1) * 512],
                                         w2s, h1s[h][:, c0:c0 + 512],
                                         start=True, stop=True)
                    drain(h2s[h][:, t * PT:(t + 1) * PT], p2[:, :], PT)

            def phi_l3_pair(h, p, slot=None):
                # tile A: m in [0,64); tile B: m in [64,128) of 16 samples.
                # ACT relus A whole-tile into scratch; one DVE STT folds
                # relu(B) + A into s (first pooling level fused into drain).
                pA = ps.tile([128, PT], f32, name="p3a", tag="hp")
                pB = ps.tile([128, PT], f32, name="p3b", tag="hp")
                for cc in range(2):
                    cA = (2 * p) * PT + cc * 512
                    nc.tensor.matmul(pA[:, cc * 512:(cc + 1) * 512],
                                     w3s, h2s[h][:, cA:cA + 512],
                                     start=True, stop=True)
                if slot is not None:
                    slot(1)
                for cc in range(2):
                    cB = (2 * p + 1) * PT + cc * 512
                    nc.tensor.matmul(pB[:, cc * 512:(cc + 1) * 512],
                                     w3s, h2s[h][:, cB:cB + 512],
                                     start=True, stop=True)
                h3ra = scr.tile([F3, PT], fp16, name="h3ra", tag="h3ra", bufs=2)
                nc.scalar.activation(h3ra[:, :], pA[0:F3, :], Relu)
                bal["act"] += 0.85 * PT + 220.0
                nc.vector.scalar_tensor_tensor(
                    s_hs[h][:, p * PT:(p + 1) * PT],
                    pB[0:F3, :], 0.0, h3ra[:, :],
                    op0=Alu.max, op1=Alu.add)
                bal["dve"] += 1.04 * PT + 190.0

            # ---- pooling trees (from 64 per sample) per 64-sample quarter;
            # whole tree on Pool (GpSimd) — it has no PSUM port so this is
            # the one big job it can absorb, freeing DVE for drains.
            def trees_q(q):
                h = q // 2
                s_sl = s_hs[h][:, (q % 2) * 4096:(q % 2) * 4096 + 4096]
                a3 = s_sl.rearrange("p (g m) -> p g m", m=64)
                t1 = scr.tile([F3, 2048], fp16, name=f"t1_{q}", tag="t1", bufs=1)
                o3 = t1[:, :].rearrange("p (g m) -> p g m", m=32)
                nc.gpsimd.tensor_tensor(out=o3, in0=a3[:, :, 0:32],
                                        in1=a3[:, :, 32:64], op=Alu.add)
                w = 32
                srt = t1[:, :]
                lv = 0
                while w > 1:
                    w //= 2
                    if w > 1:
                        nxt = scr.tile([F3, 64 * w], fp16, name=f"tl{lv}",
                                       tag=f"tl{lv}", bufs=2)[:, :]
                    else:
                        nxt = pooled[0:F3, q * QS:(q + 1) * QS]
                    a = srt.rearrange("p (g m) -> p g m", m=2 * w)
                    nc.gpsimd.tensor_tensor(
                        out=nxt.rearrange("p (g m) -> p g m", m=w) if w > 1 else nxt,
                        in0=a[:, :, 0:w], in1=a[:, :, w:2 * w], op=Alu.add)
                    srt = nxt
                    lv += 1

            # per-pair staggered tree for the last quarter: each L3 pair's 16
            # samples reduce right after their STT lands, so only the final
            # pair's short chain is exposed at the end.
            def tree_pair(h, p):
                q = 2 * h + p // 4
                s_sl = s_hs[h][:, p * PT:(p + 1) * PT]      # [80, 16*64]
                srt = s_sl
                w = 64
                lv = 0
                while w > 1:
                    w //= 2
                    if w > 1:
                        nxt = scr.tile([F3, 16 * w], fp16, name=f"pt{lv}",
                                       tag=f"pt{lv}", bufs=2)[:, :]
                    else:
                        nxt = pooled[0:F3, (p * 16) % BC + h * SAMP_HALF:
                                     (p * 16) % BC + h * SAMP_HALF + 16]
                    a = srt.rearrange("p (g m) -> p g m", m=2 * w)
                    nc.vector.tensor_tensor(
                        out=nxt.rearrange("p (g m) -> p g m", m=w) if w > 1 else nxt,
                        in0=a[:, :, 0:w], in1=a[:, :, w:2 * w], op=Alu.add)
                    srt = nxt
                    lv += 1
                bal["dve"] += 1500.0

            # ---- tail for one 64-sample quarter, emitted as a list of
            # closures so links can be interleaved between phi tiles.
            def tail_links(q):
                sl = slice(q * QS, (q + 1) * QS)
                links = []

                def l_r1():
                    pr1 = pst.tile([R1, QS], f32, name=f"pr1_{q}", tag="tail")
                    nc.tensor.matmul(pr1[:, :], r1s, pooled[:, sl],
                                     start=True, stop=True)
                    nc.scalar.activation(hr1p[0:R1, sl], pr1[:, :], Relu)

                def l_r2():
                    pr2 = pst.tile([R2, QS], f32, name=f"pr2_{q}", tag="tail")
                    nc.tensor.matmul(pr2[:, :], r2s, hr1p[:, sl],
                                     start=True, stop=True)
                    nc.scalar.activation(hr2p[0:R2, sl], pr2[:, :], Relu)

                def l_r3():
                    pr3 = pst.tile([R3, QS], f32, name=f"pr3_{q}", tag="tail")
                    nc.tensor.matmul(pr3[:, :], r3s, hr2p[:, sl],
                                     start=True, stop=True)
                    nc.scalar.copy(xqp[0:R3, sl], pr3[:, :])

                def l_q1():
                    pq1a = pst.tile([128, QS], f32, name=f"pq1a_{q}", tag="tail")
                    pq1b = pst.tile([Q1 - 128, QS], f32, name=f"pq1b_{q}", tag="tail")
                    nc.tensor.matmul(pq1a[:, :], q1as, xqp[:, sl],
                                     start=True, stop=True)
                    nc.tensor.matmul(pq1b[:, :], q1bs, xqp[:, sl],
                                     start=True, stop=True)
                    nc.scalar.activation(hq1ap[:, sl], pq1a[:, :], Relu)
                    nc.vector.tensor_scalar_max(hq1bp[0:Q1 - 128, sl],
                                                pq1b[:, :], 0.0)

                def l_q2():
                    pq2 = pst.tile([Q2, QS], f32, name=f"pq2_{q}", tag="tail")
                    nc.tensor.matmul(pq2[:, :], q2as, hq1ap[:, sl],
                                     start=True, stop=False)
                    nc.tensor.matmul(pq2[:, :], q2bs, hq1bp[:, sl],
                                     start=False, stop=True)
                    nc.scalar.activation(hq2p[0:Q2, sl], pq2[:, :], Relu)

                def l_q3():
                    pq3 = pst.tile([Q3, QS], f32, name=f"pq3_{q}", tag="tail")
                    nc.tensor.matmul(pq3[:, :], q3as, hq2p[:, sl],
                                     start=True, stop=True)
                    nc.scalar.activation(e_sbp[:, sl], pq3[:, :], Exp)

                rec = cst.tile([1, QS], f32, name=f"rec{q}")

                def l_sm1():
                    ssum = pst.tile([1, QS], f32, name=f"ssum{q}", tag="tail")
                    nc.tensor.matmul(ssum[:, :], ones3[:, :], e_sbp[:, sl],
                                     start=True, stop=True)
                    nc.vector.reciprocal(rec[:, :], ssum[:, :])

                def l_sm2():
                    eT = pst.tile([QS, 3], f32, name=f"eT{q}", tag="tail")
                    nc.tensor.transpose(eT[:, :], e_sbp[:, sl], eye3s[:, :])
                    rT = pst.tile([QS, 1], f32, name=f"rT{q}", tag="tail")
                    nc.tensor.transpose(rT[:, :], rec[:, :], eye1[:, :])
                    rTs = cst.tile([QS, 1], f32, name=f"rTs{q}")
                    nc.vector.tensor_copy(rTs[:, :], rT[:, :])
                    o_sb = cst.tile([QS, 3], f32, name=f"o_sb{q}")
                    nc.vector.tensor_scalar_mul(o_sb[:, :], eT[:, :], rTs[:, :])
                    nc.sync.dma_start(out=out[q * QS:(q + 1) * QS, :],
                                      in_=o_sb[:, :])

                links += [l_r1, l_r2, l_r3, l_q1, l_q2, l_q3, l_sm1, l_sm2]
                bal["act"] += 1500.0
                bal["dve"] += 700.0
                return links

            # ---- emission schedule ----
            pending = []               # tail links awaiting interleave slots

            def emit_pending(n=1):
                for _ in range(n):
                    if pending:
                        pending.pop(0)()

            phi_l1(0, x_sb0)
            # x half 1 DMAs early (sync ring is idle; transfers overlap L2/L3)
            x_sb1 = xp.tile([128, HALF // 4], fp16, name="x_sb1", tag="x", bufs=2)
            for j in range(4):
                nc.sync.dma_start(out=x_sb1[32 * j:32 * j + 4, :], in_=xin[1, j])
            phi_l2(0)
            for p in range(NPAIR):
                phi_l3_pair(0, p)
                if p == 3:
                    trees_q(0)
                    pending += tail_links(0)
                emit_pending(1)
            trees_q(1)
            pending += tail_links(1)
            # half 1: interleave remaining tail links 1-per-slot into the
            # L1/L2/L3 sweeps (a tail link between phi tiles hides its
            # matmul->drain round trip under the next tile's matmuls)
            phi_l1(1, x_sb1, slot=emit_pending)
            phi_l2(1, slot=emit_pending)
            for p in range(NPAIR):
                phi_l3_pair(1, p, slot=emit_pending)
                if p == 3:
                    trees_q(2)
                    pending += tail_links(2)
                if p >= 4:
                    tree_pair(1, p)      # stagger the last quarter's pooling
                emit_pending(1)
            pending += tail_links(3)
            emit_pending(len(pending))

    nc.compile()
    return nc


def _prep_inputs(dyn, static, phi_w1, phi_b1, phi_w2, phi_b2, phi_w3, phi_b3,
                 rho_w1, rho_b1, rho_w2, rho_b2, rho_w3, rho_b3,
                 q_w1, q_b1, q_w2, q_b2, q_w3, q_b3):
    """Build per-core input maps (host-side layout transforms)."""
    fp16 = np.float16

    def aug_t(w, b):
        # [out, in] weight + bias -> transposed augmented [in+1, out]
        return np.concatenate([w, b[:, None]], axis=1).T.astype(fp16)

    q1 = aug_t(q_w1, q_b1)               # [44, 200]
    q2 = aug_t(q_w2, q_b2)               # [201, 100]
    # phi weights padded to [*, 128] / [128, 128] for FWL; bias row moves to
    # row 127 (fed by the previous layer's self-computed ones row), and
    # column 127 carries a unit weight from the previous ones row so the
    # ones row propagates (h1[127] = h2[127] = 1).
    w1a = np.zeros((4, 128), dtype=fp16)
    w1a[:, 0:F1] = aug_t(phi_w1, phi_b1)
    w1a[3, 127] = 1.0                    # x ones row -> h1[127] = 1
    w2a = np.zeros((128, 128), dtype=fp16)
    w2a[0:F1, 0:F2] = phi_w2.T.astype(fp16)
    w2a[127, 0:F2] = phi_b2.astype(fp16)
    w2a[127, 127] = 1.0                  # h1 ones row -> h2[127] = 1
    w3a = np.zeros((128, 128), dtype=fp16)
    w3a[0:F2, 0:F3] = phi_w3.T.astype(fp16)
    w3a[127, 0:F3] = phi_b3.astype(fp16)
    parts = dict(
        w1a=w1a, w2a=w2a, w3a=w3a, r1a=aug_t(rho_w1, rho_b1),
        r2a=aug_t(rho_w2, rho_b2), r3a=aug_t(rho_w3, rho_b3),
        q1aw=q1[:, 0:128], q1bw=q1[:, 128:],
        q2aw=q2[0:128, :], q2bw=q2[128:, :], q3aw=aug_t(q_w3, q_b3))

    eye3 = np.eye(3, dtype=np.float32)
    onesr = np.ones((1, HALF), dtype=fp16)

    base_blob = np.zeros((128, BLOBW), dtype=fp16)
    for name, (r, cc, o) in _BLOB.items():
        if name != "statt":
            base_blob[0:r, o:o + cc] = parts[name]
    for j in range(1, 4):   # replicate L1 weights into each PE row-group
        base_blob[32 * j:32 * j + 4, 0:128] = parts["w1a"]

    # element order within a half: tile T (1024 cols), pair p = T//2:
    # tile A (T even): col c -> sample p*16 + c//64, m = c%64
    # tile B (T odd):  col c -> sample p*16 + c//64, m = 64 + c%64
    # Build permutation: halfcol g -> (sample_in_half, m)
    Tl = np.arange(HALF) // PT
    cl = np.arange(HALF) % PT
    pl = Tl // 2
    samp = pl * 16 + cl // 64
    mm = (Tl % 2) * 64 + cl % 64
    elem_of_col = samp * M + mm          # index into half's [2048*M?] no: per-half

    in_maps = []
    for c in range(N_CORES):
        blob = base_blob.copy()
        r, cc, o = _BLOB["statt"]
        blob[0:r, o:o + cc] = static[c * BC:(c + 1) * BC].T.astype(fp16)
        xc = dyn[c * BC:(c + 1) * BC].reshape(EC, D)
        # xin[h, j, row, t*256 + n] = comp(row) of element at tile t, block j
        xin = np.empty((2, 4, 4, HALF // 4), dtype=fp16)
        for hh in range(2):
            xh = xc[hh * HALF:(hh + 1) * HALF]          # [16384, 3]
            xperm = xh[elem_of_col]                      # cols in emission order
            comp = np.concatenate([xperm, np.ones((HALF, 1), np.float32)],
                                  axis=1)                # [HALF, 4] comps
            # blocks of L1B cols RR over groups: block b -> group b%4,
            # group-local slot b//4
            blocks = comp.reshape(HALF // L1B, L1B, 4)
            for j in range(4):
                grp = blocks[j::4]                   # [HALF//L1B//4, L1B, 4]
                xin[hh, j] = grp.transpose(2, 0, 1).reshape(4, HALF // 4)
        in_maps.append(dict(xin=xin, blob=blob, onesr=onesr, eye3=eye3))
    return in_maps


def kernel(**inputs):
    import time
    from concourse.bass_utils import run_bass_kernel_spmd

    if "nc" not in _compiled:
        _compiled["nc"] = _build()
    nc = _compiled["nc"]

    in_maps = _prep_inputs(**inputs)
    last_err = None
    for attempt in range(3):
        try:
            res = run_bass_kernel_spmd(nc, in_maps, core_ids=list(range(N_CORES)))
            break
        except Exception as e:          # transient device errors: back off and retry
            last_err = e
            time.sleep(20 * (attempt + 1))
    else:
        raise last_err
    out = np.concatenate([res.results[c]["out"] for c in range(N_CORES)], axis=0)
    return out.astype(np.float32)


# revision 42
# speedup vs baseline: 1.0159x; 1.0036x over previous
"""Trainium2 Bass kernel for the DeepSets-style segment_reduce network.

Network (per sample, B=2048, M=128 elements):
  phi: 3 -> 120 -> 100 -> 80 MLP (all ReLU), applied per element
  pooled = sum over the 128 elements                      [B, 80]
  rho:  80 -> 60 -> 60 -> 40 (ReLU, ReLU, linear)
  q:    concat(rho_out, static) 43 -> 200 -> 100 -> 3, softmax

Mapping: data-parallel over 8 NeuronCores (256 samples each).
v2 design (from trace analysis of the v1 kernel):
  - L1 runs as fp8e4 DoubleRow matmuls (0.5 cycles/col) packed 4-way across
    PE row-groups; x is host-packed into the [2-part, 2-subtile] DR layout.
  - L2/L3 stay fp16 (1 cycle/col), stationary weights, 512-col moving ops.
  - PSUM drain (the real wall: ~98k columns through ACT+DVE at 1/cycle):
    L1/L2 tiles are whole-tile relu drains assigned to ACT or DVE by a
    load-balance heuristic; L3 drains use *tile-pair folding*: element
    order pairs tile A (m<64) with tile B (m>=64) column-for-column, so
    ACT relus tile A whole, and one whole-tile DVE STT does
    relu(tileB)+tileA — the first pooling level fused into the drain.
  - Pool (GpSimd) has no PSUM port; it takes tree level-1 (fp16 SBUF adds).
  - Tails run per 64-sample quarter, their serial matmul->drain links
    interleaved between phi tile emissions so the PE never idles on the
    chain; only the last quarter's tree+tail chain is exposed.
  - x DMA is chunked across both HWDGE rings (sync+scalar) so the first
    L1 matmul is not gated by a 5us serial DMA; the PE warmup bridges
    directly into L1 to keep the HAM/p-state ramp alive.
"""

import sys
import numpy as np

sys.path.insert(0, '/opt/trn_rl_repo')

B, M, D = 2048, 128, 3
N_CORES = 8
BC = B // N_CORES            # samples per core (256)
EC = BC * M                  # elements per core (32768)
HALF = EC // 2               # elements per half (16384), 128 samples
SAMP_HALF = BC // 2          # samples per half (128)
QS = 64                      # tail quarter size (samples)
PT = 1024                    # cols per PSUM tile
TPH = HALF // PT             # psum tiles per half (16)
NPAIR = TPH // 2             # L3 tile pairs per half (8)
L1B = 512                    # L1 matmul block width (cols)

F1, F2, F3 = 120, 100, 80    # phi widths
R1, R2, R3 = 60, 60, 40      # rho widths
Q1, Q2, Q3 = 200, 100, 3     # q widths
XQ = R3 + 3 + 1              # q input rows: rho_out + static + ones (44)

# fp16 weight blob column layout: name -> (rows, cols, col_offset)
# phi weights are padded to K=128/M=128 so FWL (fast weight load) triggers;
# a unit weight on the previous layer's ones-row makes each layer's ones-row
# self-compute in PSUM (h1[127]=1, h2[127]=1), so no ones-row DMAs needed.
_BLOB = {}
_off = 0
for _name, _r, _c in [("w1a", 4, 128), ("w2a", 128, 128), ("w3a", 128, 128),
                      ("r1a", F3 + 1, R1), ("r2a", R1 + 1, R2), ("r3a", R2 + 1, R3),
                      ("q1aw", XQ, 128), ("q1bw", XQ, Q1 - 128),
                      ("q2aw", 128, Q2), ("q2bw", Q1 - 128 + 1, Q2),
                      ("q3aw", Q2 + 1, Q3), ("statt", 3, BC)]:
    _BLOB[_name] = (_r, _c, _off)
    _off += _c
BLOBW = _off

_compiled = {}


def _build():
    import concourse.bacc as bacc
    import concourse.mybir as mybir
    from concourse import tile

    f32 = mybir.dt.float32
    fp16 = mybir.dt.float16
    Relu = mybir.ActivationFunctionType.Relu
    Exp = mybir.ActivationFunctionType.Exp
    Alu = mybir.AluOpType

    nc = bacc.Bacc("TRN2", target_bir_lowering=False, debug=False,
                   enable_asserts=False, num_devices=N_CORES)

    # x: [half, group, row(x0,x1,x2,ones), cols] fp16, 256-col blocks RR
    xin = nc.dram_tensor("xin", [2, 4, 4, HALF // 4], fp16,
                         kind="ExternalInput").ap()
    blob = nc.dram_tensor("blob", [128, BLOBW], fp16, kind="ExternalInput").ap()
    onesr = nc.dram_tensor("onesr", [1, HALF], fp16, kind="ExternalInput").ap()
    eye3 = nc.dram_tensor("eye3", [3, 3], f32, kind="ExternalInput").ap()
    out = nc.dram_tensor("out", [BC, 3], f32, kind="ExternalOutput").ap()

    # drain engine load-balance state (emit-time estimates, ns)
    bal = {"act": 0.0, "dve": 0.0}

    with tile.TileContext(nc) as tc:
        with tc.tile_pool(name="cst", bufs=1) as cst, \
             tc.tile_pool(name="xp", bufs=2) as xp, \
             tc.tile_pool(name="scr", bufs=1) as scr, \
             tc.tile_pool(name="ps", bufs=3, space="PSUM") as ps, \
             tc.tile_pool(name="pst", bufs=2, space="PSUM") as pst:

            # ---- input DMAs. HWDGE transfers run FIFO per ring, so order by
            # need: w1a first (gates L1), then x half-0 split across both
            # rings, then the rest of the weights.
            blob_sb = cst.tile([128, BLOBW], fp16)
            nc.scalar.dma_start(out=blob_sb[:, 0:128], in_=blob[:, 0:128])
            x_sb0 = xp.tile([128, HALF // 4], fp16, name="x_sb0", tag="x", bufs=2)
            # spread x groups over the 3 DMA-capable rings: completion
            # latency (~2us each) is FIFO-serialized per ring
            for j, eng in enumerate((nc.sync, nc.scalar, nc.gpsimd, nc.sync)):
                eng.dma_start(out=x_sb0[32 * j:32 * j + 4, :], in_=xin[0, j])
            PHI_W_COLS = 384            # w1a+w2a+w3a
            nc.scalar.dma_start(out=blob_sb[:, 128:PHI_W_COLS],
                                in_=blob[:, 128:PHI_W_COLS])
            nc.sync.dma_start(out=blob_sb[:, PHI_W_COLS:],
                              in_=blob[:, PHI_W_COLS:])

            def wslice(name):
                r, c, o = _BLOB[name]
                return blob_sb[0:r, o:o + c]

            w2s, w3s = wslice("w2a"), wslice("w3a")
            r1s, r2s, r3s = wslice("r1a"), wslice("r2a"), wslice("r3a")
            q1as, q1bs = wslice("q1aw"), wslice("q1bw")
            q2as, q2bs = wslice("q2aw"), wslice("q2bw")
            q3as, statt = wslice("q3aw"), wslice("statt")

            # PE warm-up source
            wtiny = cst.tile([128, 512], fp16)
            nc.vector.memset(wtiny[:, :], 0.0)

            # persistent activation planes (ones rows self-compute via the
            # padded weight columns; no ones DMAs needed)
            h1s, h2s, s_hs = [], [], []
            for h in range(2):
                h1s.append(cst.tile([128, HALF], fp16, name=f"h1_{h}"))
                h2s.append(cst.tile([128, HALF], fp16, name=f"h2_{h}"))
                s_hs.append(cst.tile([F3, HALF // 2], fp16, name=f"s_{h}"))

            pooled = cst.tile([F3 + 1, BC], fp16)      # ones row at 80
            nc.gpsimd.dma_start(out=pooled[F3:F3 + 1, :], in_=onesr[:, 0:BC])
            eye3s = cst.tile([3, 3], f32)
            nc.gpsimd.dma_start(out=eye3s[:, :], in_=eye3)
            eye1 = cst.tile([1, 1], f32)
            nc.vector.memset(eye1[:, :], 1.0)
            ones3 = cst.tile([3, 1], f32)
            nc.vector.memset(ones3[:, :], 1.0)

            # persistent tail planes [*, BC] (ones rows written once)
            xqp = cst.tile([XQ, BC], fp16)
            _r, _c, _o = _BLOB["statt"]
            nc.gpsimd.dma_start(out=xqp[R3:R3 + 3, :], in_=blob[0:3, _o:_o + _c])
            nc.gpsimd.dma_start(out=xqp[XQ - 1:XQ, :], in_=onesr[:, 0:BC])
            hr1p = cst.tile([R1 + 1, BC], fp16)
            nc.gpsimd.dma_start(out=hr1p[R1:R1 + 1, :], in_=onesr[:, 0:BC])
            hr2p = cst.tile([R2 + 1, BC], fp16)
            nc.gpsimd.dma_start(out=hr2p[R2:R2 + 1, :], in_=onesr[:, 0:BC])
            hq1ap = cst.tile([128, BC], fp16)
            hq1bp = cst.tile([Q1 - 128 + 1, BC], fp16)
            nc.gpsimd.dma_start(out=hq1bp[Q1 - 128:Q1 - 128 + 1, :],
                                in_=onesr[:, 0:BC])
            hq2p = cst.tile([Q2 + 1, BC], fp16)
            nc.gpsimd.dma_start(out=hq2p[Q2:Q2 + 1, :], in_=onesr[:, 0:BC])
            e_sbp = cst.tile([3, BC], f32)
            o_all = cst.tile([QS, 12], f32)      # [sample, quarter*3+c] staging

            # warm-up matmuls: back-to-back, bridging into L1 (ramp the PE)
            pw = ps.tile([128, 512], f32, name="pw", tag="hp")
            for i in range(4):
                nc.tensor.matmul(pw[:, :], wtiny[:, 0:128], wtiny[:, :],
                                 start=(i == 0), stop=(i == 3))
            wsink = cst.tile([32, 2], fp16)
            nc.vector.tensor_scalar_max(wsink[:, :], pw[0:32, 0:2], 0.0)

            # ---- drain helper: whole-tile relu drain on ACT or DVE by balance
            def drain(dst, src, ncols):
                est_a = 0.85 * ncols + 220.0
                est_d = 1.04 * ncols + 190.0
                if bal["act"] + est_a <= bal["dve"] + est_d:
                    bal["act"] += est_a
                    nc.scalar.activation(dst, src, Relu)
                else:
                    bal["dve"] += est_d
                    nc.vector.tensor_scalar_max(dst, src, 0.0)

            # ---- phi layer sweeps ----
            def phi_l1(h, x_sb, slot=None):
                # L1 blocks of L1B cols round-robin over PE row-groups
                bpt = PT // L1B
                for t in range(TPH):
                    if slot is not None:
                        slot(1)
                    p1 = ps.tile([128, PT], f32, name="p1", tag="hp")
                    for cc in range(bpt):
                        b = bpt * t + cc
                        j = b % 4
                        bl = b // 4
                        nc.tensor.matmul(
                            p1[:, cc * L1B:(cc + 1) * L1B],
                            blob_sb[32 * j:32 * j + 4, 0:128],
                            x_sb[32 * j:32 * j + 4, bl * L1B:(bl + 1) * L1B],
                            start=True, stop=True, tile_position=(32 * j, 0))
                    drain(h1s[h][:, t * PT:(t + 1) * PT], p1[:, :], PT)

            def phi_l2(h, slot=None):
                for t in range(TPH):
                    if slot is not None:
                        slot(1)
                    p2 = ps.tile([128, PT], f32, name="p2", tag="hp")
                    for cc in range(2):
                        c0 = t * PT + cc * 512
                        nc.tensor.matmul(p2[:, cc * 512:(cc + 1) * 512],
                                         w2s, h1s[h][:, c0:c0 + 512],
                                         start=True, stop=True)
                    drain(h2s[h][:, t * PT:(t + 1) * PT], p2[:, :], PT)

            def phi_l3_pair(h, p, slot=None):
                # tile A: m in [0,64); tile B: m in [64,128) of 16 samples.
                # ACT relus A whole-tile into scratch; one DVE STT folds
                # relu(B) + A into s (first pooling level fused into drain).
                pA = ps.tile([128, PT], f32, name="p3a", tag="hp")
                pB = ps.tile([128, PT], f32, name="p3b", tag="hp")
                for cc in range(2):
                    cA = (2 * p) * PT + cc * 512
                    nc.tensor.matmul(pA[:, cc * 512:(cc + 1) * 512],
                                     w3s, h2s[h][:, cA:cA + 512],
                                     start=True, stop=True)
                if slot is not None:
                    slot(1)
                for cc in range(2):
                    cB = (2 * p + 1) * PT + cc * 512
                    nc.tensor.matmul(pB[:, cc * 512:(cc + 1) * 512],
                                     w3s, h2s[h][:, cB:cB + 512],
                                     start=True, stop=True)
                h3ra = scr.tile([F3, PT], fp16, name="h3ra", tag="h3ra", bufs=2)
                nc.scalar.activation(h3ra[:, :], pA[0:F3, :], Relu)
                bal["act"] += 0.85 * PT + 220.0
                nc.vector.scalar_tensor_tensor(
                    s_hs[h][:, p * PT:(p + 1) * PT],
                    pB[0:F3, :], 0.0, h3ra[:, :],
                    op0=Alu.max, op1=Alu.add)
                bal["dve"] += 1.04 * PT + 190.0

            # ---- pooling trees (from 64 per sample) per 64-sample quarter;
            # whole tree on Pool (GpSimd) — it has no PSUM port so this is
            # the one big job it can absorb, freeing DVE for drains.
            def trees_q(q):
                h = q // 2
                s_sl = s_hs[h][:, (q % 2) * 4096:(q % 2) * 4096 + 4096]
                a3 = s_sl.rearrange("p (g m) -> p g m", m=64)
                t1 = scr.tile([F3, 2048], fp16, name=f"t1_{q}", tag="t1", bufs=1)
                o3 = t1[:, :].rearrange("p (g m) -> p g m", m=32)
                nc.gpsimd.tensor_tensor(out=o3, in0=a3[:, :, 0:32],
                                        in1=a3[:, :, 32:64], op=Alu.add)
                w = 32
                srt = t1[:, :]
                lv = 0
                while w > 1:
                    w //= 2
                    if w > 1:
                        nxt = scr.tile([F3, 64 * w], fp16, name=f"tl{lv}",
                                       tag=f"tl{lv}", bufs=2)[:, :]
                    else:
                        nxt = pooled[0:F3, q * QS:(q + 1) * QS]
                    a = srt.rearrange("p (g m) -> p g m", m=2 * w)
                    nc.gpsimd.tensor_tensor(
                        out=nxt.rearrange("p (g m) -> p g m", m=w) if w > 1 else nxt,
                        in0=a[:, :, 0:w], in1=a[:, :, w:2 * w], op=Alu.add)
                    srt = nxt
                    lv += 1

            # per-pair staggered tree for the last quarter: each L3 pair's 16
            # samples reduce right after their STT lands, so only the final
            # pair's short chain is exposed at the end.
            def tree_pair(h, p):
                q = 2 * h + p // 4
                s_sl = s_hs[h][:, p * PT:(p + 1) * PT]      # [80, 16*64]
                srt = s_sl
                w = 64
                lv = 0
                while w > 1:
                    w //= 2
                    if w > 1:
                        nxt = scr.tile([F3, 16 * w], fp16, name=f"pt{lv}",
                                       tag=f"pt{lv}", bufs=2)[:, :]
                    else:
                        nxt = pooled[0:F3, (p * 16) % BC + h * SAMP_HALF:
                                     (p * 16) % BC + h * SAMP_HALF + 16]
                    a = srt.rearrange("p (g m) -> p g m", m=2 * w)
                    nc.vector.tensor_tensor(
                        out=nxt.rearrange("p (g m) -> p g m", m=w) if w > 1 else nxt,
                        in0=a[:, :, 0:w], in1=a[:, :, w:2 * w], op=Alu.add)
                    srt = nxt
                    lv += 1
                bal["dve"] += 1500.0

            # ---- tail for one 64-sample quarter, emitted as a list of
            # closures so links can be interleaved between phi tiles.
            def tail_links(q):
                sl = slice(q * QS, (q + 1) * QS)
                links = []

                def l_r1():
                    pr1 = pst.tile([R1, QS], f32, name=f"pr1_{q}", tag="tail")
                    nc.tensor.matmul(pr1[:, :], r1s, pooled[:, sl],
                                     start=True, stop=True)
                    nc.scalar.activation(hr1p[0:R1, sl], pr1[:, :], Relu)

                def l_r2():
                    pr2 = pst.tile([R2, QS], f32, name=f"pr2_{q}", tag="tail")
                    nc.tensor.matmul(pr2[:, :], r2s, hr1p[:, sl],
                                     start=True, stop=True)
                    nc.scalar.activation(hr2p[0:R2, sl], pr2[:, :], Relu)

                def l_r3():
                    pr3 = pst.tile([R3, QS], f32, name=f"pr3_{q}", tag="tail")
                    nc.tensor.matmul(pr3[:, :], r3s, hr2p[:, sl],
                                     start=True, stop=True)
                    nc.scalar.copy(xqp[0:R3, sl], pr3[:, :])

                def l_q1():
                    pq1a = pst.tile([128, QS], f32, name=f"pq1a_{q}", tag="tail")
                    pq1b = pst.tile([Q1 - 128, QS], f32, name=f"pq1b_{q}", tag="tail")
                    nc.tensor.matmul(pq1a[:, :], q1as, xqp[:, sl],
                                     start=True, stop=True)
                    nc.tensor.matmul(pq1b[:, :], q1bs, xqp[:, sl],
                                     start=True, stop=True)
                    nc.scalar.activation(hq1ap[:, sl], pq1a[:, :], Relu)
                    nc.vector.tensor_scalar_max(hq1bp[0:Q1 - 128, sl],
                                                pq1b[:, :], 0.0)

                def l_q2():
                    pq2 = pst.tile([Q2, QS], f32, name=f"pq2_{q}", tag="tail")
                    nc.tensor.matmul(pq2[:, :], q2as, hq1ap[:, sl],
                                     start=True, stop=False)
                    nc.tensor.matmul(pq2[:, :], q2bs, hq1bp[:, sl],
                                     start=False, stop=True)
                    nc.scalar.activation(hq2p[0:Q2, sl], pq2[:, :], Relu)

                def l_q3():
                    pq3 = pst.tile([Q3, QS], f32, name=f"pq3_{q}", tag="tail")
                    nc.tensor.matmul(pq3[:, :], q3as, hq2p[:, sl],
                                     start=True, stop=True)
                    nc.scalar.activation(e_sbp[:, sl], pq3[:, :], Exp)

                rec = cst.tile([1, QS], f32, name=f"rec{q}")

                def l_sm1():
                    ssum = pst.tile([1, QS], f32, name=f"ssum{q}", tag="tail")
                    nc.tensor.matmul(ssum[:, :], ones3[:, :], e_sbp[:, sl],
                                     start=True, stop=True)
                    nc.vector.reciprocal(rec[:, :], ssum[:, :])

                def l_sm2():
                    eT = pst.tile([QS, 3], f32, name=f"eT{q}", tag="tail")
                    nc.tensor.transpose(eT[:, :], e_sbp[:, sl], eye3s[:, :])
                    rT = pst.tile([QS, 1], f32, name=f"rT{q}", tag="tail")
                    nc.tensor.transpose(rT[:, :], rec[:, :], eye1[:, :])
                    rTs = cst.tile([QS, 1], f32, name=f"rTs{q}")
                    nc.vector.tensor_copy(rTs[:, :], rT[:, :])
                    nc.vector.tensor_scalar_mul(o_all[:, 3 * q:3 * q + 3],
                                                eT[:, :], rTs[:, :])

                links += [l_r1, l_r2, l_r3, l_q1, l_q2, l_q3, l_sm1, l_sm2]
                bal["act"] += 1500.0
                bal["dve"] += 700.0
                return links

            # ---- emission schedule ----
            pending = []               # tail links awaiting interleave slots

            def emit_pending(n=1):
                for _ in range(n):
                    if pending:
                        pending.pop(0)()

            phi_l1(0, x_sb0)
            # x half 1 DMAs early (sync ring is idle; transfers overlap L2/L3)
            x_sb1 = xp.tile([128, HALF // 4], fp16, name="x_sb1", tag="x", bufs=2)
            for j in range(4):
                nc.sync.dma_start(out=x_sb1[32 * j:32 * j + 4, :], in_=xin[1, j])
            phi_l2(0)
            for p in range(NPAIR):
                phi_l3_pair(0, p)
                if p == 3:
                    trees_q(0)
                    pending += tail_links(0)
                emit_pending(1)
            trees_q(1)
            pending += tail_links(1)
            # half 1: interleave remaining tail links 1-per-slot into the
            # L1/L2/L3 sweeps (a tail link between phi tiles hides its
            # matmul->drain round trip under the next tile's matmuls)
            phi_l1(1, x_sb1, slot=emit_pending)
            phi_l2(1, slot=emit_pending)
            for p in range(NPAIR):
                phi_l3_pair(1, p, slot=emit_pending)
                if p == 3:
                    trees_q(2)
                    pending += tail_links(2)
                if p >= 4:
                    tree_pair(1, p)      # stagger the last quarter's pooling
                emit_pending(1)
            pending += tail_links(3)
            emit_pending(len(pending))
            # single output DMA: one completion-latency chain instead of four
            nc.sync.dma_start(
                out=out.rearrange("(q s) c -> s q c", q=4),
                in_=o_all[:, :].rearrange("p (q c) -> p q c", q=4))

    nc.compile()
    return nc


def _prep_inputs(dyn, static, phi_w1, phi_b1, phi_w2, phi_b2, phi_w3, phi_b3,
                 rho_w1, rho_b1, rho_w2, rho_b2, rho_w3, rho_b3,
                 q_w1, q_b1, q_w2, q_b2, q_w3, q_b3):
    """Build per-core input maps (host-side layout transforms)."""
    fp16 = np.float16

    def aug_t(w, b):
        # [out, in] weight + bias -> transposed augmented [in+1, out]
        return np.concatenate([w, b[:, None]], axis=1).T.astype(fp16)

    q1 = aug_t(q_w1, q_b1)               # [44, 200]
    q2 = aug_t(q_w2, q_b2)               # [201, 100]
    # phi weights padded to [*, 128] / [128, 128] for FWL; bias row moves to
    # row 127 (fed by the previous layer's self-computed ones row), and
    # column 127 carries a unit weight from the previous ones row so the
    # ones row propagates (h1[127] = h2[127] = 1).
    w1a = np.zeros((4, 128), dtype=fp16)
    w1a[:, 0:F1] = aug_t(phi_w1, phi_b1)
    w1a[3, 127] = 1.0                    # x ones row -> h1[127] = 1
    w2a = np.zeros((128, 128), dtype=fp16)
    w2a[0:F1, 0:F2] = phi_w2.T.astype(fp16)
    w2a[127, 0:F2] = phi_b2.astype(fp16)
    w2a[127, 127] = 1.0                  # h1 ones row -> h2[127] = 1
    w3a = np.zeros((128, 128), dtype=fp16)
    w3a[0:F2, 0:F3] = phi_w3.T.astype(fp16)
    w3a[127, 0:F3] = phi_b3.astype(fp16)
    parts = dict(
        w1a=w1a, w2a=w2a, w3a=w3a, r1a=aug_t(rho_w1, rho_b1),
        r2a=aug_t(rho_w2, rho_b2), r3a=aug_t(rho_w3, rho_b3),
        q1aw=q1[:, 0:128], q1bw=q1[:, 128:],
        q2aw=q2[0:128, :], q2bw=q2[128:, :], q3aw=aug_t(q_w3, q_b3))

    eye3 = np.eye(3, dtype=np.float32)
    onesr = np.ones((1, HALF), dtype=fp16)

    base_blob = np.zeros((128, BLOBW), dtype=fp16)
    for name, (r, cc, o) in _BLOB.items():
        if name != "statt":
            base_blob[0:r, o:o + cc] = parts[name]
    for j in range(1, 4):   # replicate L1 weights into each PE row-group
        base_blob[32 * j:32 * j + 4, 0:128] = parts["w1a"]

    # element order within a half: tile T (1024 cols), pair p = T//2:
    # tile A (T even): col c -> sample p*16 + c//64, m = c%64
    # tile B (T odd):  col c -> sample p*16 + c//64, m = 64 + c%64
    # Build permutation: halfcol g -> (sample_in_half, m)
    Tl = np.arange(HALF) // PT
    cl = np.arange(HALF) % PT
    pl = Tl // 2
    samp = pl * 16 + cl // 64
    mm = (Tl % 2) * 64 + cl % 64
    elem_of_col = samp * M + mm          # index into half's [2048*M?] no: per-half

    in_maps = []
    for c in range(N_CORES):
        blob = base_blob.copy()
        r, cc, o = _BLOB["statt"]
        blob[0:r, o:o + cc] = static[c * BC:(c + 1) * BC].T.astype(fp16)
        xc = dyn[c * BC:(c + 1) * BC].reshape(EC, D)
        # xin[h, j, row, t*256 + n] = comp(row) of element at tile t, block j
        xin = np.empty((2, 4, 4, HALF // 4), dtype=fp16)
        for hh in range(2):
            xh = xc[hh * HALF:(hh + 1) * HALF]          # [16384, 3]
            xperm = xh[elem_of_col]                      # cols in emission order
            comp = np.concatenate([xperm, np.ones((HALF, 1), np.float32)],
                                  axis=1)                # [HALF, 4] comps
            # blocks of L1B cols RR over groups: block b -> group b%4,
            # group-local slot b//4
            blocks = comp.reshape(HALF // L1B, L1B, 4)
            for j in range(4):
                grp = blocks[j::4]                   # [HALF//L1B//4, L1B, 4]
                xin[hh, j] = grp.transpose(2, 0, 1).reshape(4, HALF // 4)
        in_maps.append(dict(xin=xin, blob=blob, onesr=onesr, eye3=eye3))
    return in_maps


def kernel(**inputs):
    import time
    from concourse.bass_utils import run_bass_kernel_spmd

    if "nc" not in _compiled:
        _compiled["nc"] = _build()
    nc = _compiled["nc"]

    in_maps = _prep_inputs(**inputs)
    last_err = None
    for attempt in range(3):
        try:
            res = run_bass_kernel_spmd(nc, in_maps, core_ids=list(range(N_CORES)))
            break
        except Exception as e:          # transient device errors: back off and retry
            last_err = e
            time.sleep(20 * (attempt + 1))
    else:
        raise last_err
    out = np.concatenate([res.results[c]["out"] for c in range(N_CORES)], axis=0)
    return out.astype(np.float32)


# revision 46
# speedup vs baseline: 1.0446x; 1.0282x over previous
"""Trainium2 Bass kernel for the DeepSets-style segment_reduce network.

Network (per sample, B=2048, M=128 elements):
  phi: 3 -> 120 -> 100 -> 80 MLP (all ReLU), applied per element
  pooled = sum over the 128 elements                      [B, 80]
  rho:  80 -> 60 -> 60 -> 40 (ReLU, ReLU, linear)
  q:    concat(rho_out, static) 43 -> 200 -> 100 -> 3, softmax

Mapping: data-parallel over 8 NeuronCores (256 samples each).
v2 design (from trace analysis of the v1 kernel):
  - L1 runs as fp8e4 DoubleRow matmuls (0.5 cycles/col) packed 4-way across
    PE row-groups; x is host-packed into the [2-part, 2-subtile] DR layout.
  - L2/L3 stay fp16 (1 cycle/col), stationary weights, 512-col moving ops.
  - PSUM drain (the real wall: ~98k columns through ACT+DVE at 1/cycle):
    L1/L2 tiles are whole-tile relu drains assigned to ACT or DVE by a
    load-balance heuristic; L3 drains use *tile-pair folding*: element
    order pairs tile A (m<64) with tile B (m>=64) column-for-column, so
    ACT relus tile A whole, and one whole-tile DVE STT does
    relu(tileB)+tileA — the first pooling level fused into the drain.
  - Pool (GpSimd) has no PSUM port; it takes tree level-1 (fp16 SBUF adds).
  - Tails run per 64-sample quarter, their serial matmul->drain links
    interleaved between phi tile emissions so the PE never idles on the
    chain; only the last quarter's tree+tail chain is exposed.
  - x DMA is chunked across both HWDGE rings (sync+scalar) so the first
    L1 matmul is not gated by a 5us serial DMA; the PE warmup bridges
    directly into L1 to keep the HAM/p-state ramp alive.
"""

import sys
import numpy as np

sys.path.insert(0, '/opt/trn_rl_repo')

B, M, D = 2048, 128, 3
N_CORES = 8
BC = B // N_CORES            # samples per core (256)
EC = BC * M                  # elements per core (32768)
HALF = EC // 2               # elements per half (16384), 128 samples
SAMP_HALF = BC // 2          # samples per half (128)
QS = 64                      # tail quarter size (samples)
PT = 1024                    # cols per PSUM tile
TPH = HALF // PT             # psum tiles per half (16)
NPAIR = TPH // 2             # L3 tile pairs per half (8)
L1B = 512                    # L1 matmul block width (cols)

F1, F2, F3 = 120, 100, 80    # phi widths
R1, R2, R3 = 60, 60, 40      # rho widths
Q1, Q2, Q3 = 200, 100, 3     # q widths
XQ = R3 + 3 + 1              # q input rows: rho_out + static + ones (44)

# fp16 weight blob column layout: name -> (rows, cols, col_offset)
# phi weights are padded to K=128/M=128 so FWL (fast weight load) triggers;
# a unit weight on the previous layer's ones-row makes each layer's ones-row
# self-compute in PSUM (h1[127]=1, h2[127]=1), so no ones-row DMAs needed.
_BLOB = {}
_off = 0
for _name, _r, _c in [("w1a", 4, 128), ("w2a", 128, 128), ("w3a", 128, 128),
                      ("r1a", F3 + 1, R1), ("r2a", R1 + 1, R2), ("r3a", R2 + 1, R3),
                      ("q1aw", XQ, 128), ("q1bw", XQ, Q1 - 128),
                      ("q2aw", 128, Q2), ("q2bw", Q1 - 128 + 1, Q2),
                      ("q3aw", Q2 + 1, Q3), ("statt", 3, BC)]:
    _BLOB[_name] = (_r, _c, _off)
    _off += _c
BLOBW = _off

_compiled = {}


def _build():
    import concourse.bacc as bacc
    import concourse.mybir as mybir
    from concourse import tile

    f32 = mybir.dt.float32
    fp16 = mybir.dt.float16
    Relu = mybir.ActivationFunctionType.Relu
    Exp = mybir.ActivationFunctionType.Exp
    Alu = mybir.AluOpType

    nc = bacc.Bacc("TRN2", target_bir_lowering=False, debug=False,
                   enable_asserts=False, num_devices=N_CORES)

    # x: [half, group, row(x0,x1,x2,ones), cols] fp16, 256-col blocks RR
    xin = nc.dram_tensor("xin", [2, 4, 4, HALF // 4], fp16,
                         kind="ExternalInput").ap()
    blob = nc.dram_tensor("blob", [128, BLOBW], fp16, kind="ExternalInput").ap()
    onesr = nc.dram_tensor("onesr", [1, HALF], fp16, kind="ExternalInput").ap()
    eye3 = nc.dram_tensor("eye3", [3, 3], f32, kind="ExternalInput").ap()
    out = nc.dram_tensor("out", [BC, 3], f32, kind="ExternalOutput").ap()

    # drain engine load-balance state (emit-time estimates, ns)
    bal = {"act": 0.0, "dve": 0.0}

    with tile.TileContext(nc) as tc:
        with tc.tile_pool(name="cst", bufs=1) as cst, \
             tc.tile_pool(name="xp", bufs=2) as xp, \
             tc.tile_pool(name="scr", bufs=1) as scr, \
             tc.tile_pool(name="ps", bufs=3, space="PSUM") as ps, \
             tc.tile_pool(name="pst", bufs=2, space="PSUM") as pst:

            # ---- input DMAs. HWDGE transfers run FIFO per ring, so order by
            # need: w1a first (gates L1), then x half-0 split across both
            # rings, then the rest of the weights.
            blob_sb = cst.tile([128, BLOBW], fp16)
            nc.scalar.dma_start(out=blob_sb[:, 0:128], in_=blob[:, 0:128])
            x_sb0 = xp.tile([128, HALF // 4], fp16, name="x_sb0", tag="x", bufs=2)
            # spread x groups over the 3 DMA-capable rings: completion
            # latency (~2us each) is FIFO-serialized per ring
            for j, eng in enumerate((nc.sync, nc.scalar, nc.gpsimd, nc.sync)):
                eng.dma_start(out=x_sb0[32 * j:32 * j + 4, :], in_=xin[0, j])
            PHI_W_COLS = 384            # w1a+w2a+w3a
            nc.sync.dma_start(out=blob_sb[:, 128:PHI_W_COLS],
                              in_=blob[:, 128:PHI_W_COLS])
            nc.sync.dma_start(out=blob_sb[:, PHI_W_COLS:],
                              in_=blob[:, PHI_W_COLS:])

            def wslice(name):
                r, c, o = _BLOB[name]
                return blob_sb[0:r, o:o + c]

            w2s, w3s = wslice("w2a"), wslice("w3a")
            r1s, r2s, r3s = wslice("r1a"), wslice("r2a"), wslice("r3a")
            q1as, q1bs = wslice("q1aw"), wslice("q1bw")
            q2as, q2bs = wslice("q2aw"), wslice("q2bw")
            q3as, statt = wslice("q3aw"), wslice("statt")

            # PE warm-up source
            wtiny = cst.tile([128, 512], fp16)
            nc.vector.memset(wtiny[:, :], 0.0)

            # persistent activation planes (ones rows self-compute via the
            # padded weight columns; no ones DMAs needed)
            h1s, h2s, s_hs = [], [], []
            for h in range(2):
                h1s.append(cst.tile([128, HALF], fp16, name=f"h1_{h}"))
                h2s.append(cst.tile([128, HALF], fp16, name=f"h2_{h}"))
                s_hs.append(cst.tile([F3, HALF // 2], fp16, name=f"s_{h}"))

            pooled = cst.tile([F3 + 1, BC], fp16)      # ones row at 80
            nc.gpsimd.dma_start(out=pooled[F3:F3 + 1, :], in_=onesr[:, 0:BC])
            eye3s = cst.tile([3, 3], f32)
            nc.gpsimd.dma_start(out=eye3s[:, :], in_=eye3)
            eye1 = cst.tile([1, 1], f32)
            nc.vector.memset(eye1[:, :], 1.0)
            ones3 = cst.tile([3, 1], f32)
            nc.vector.memset(ones3[:, :], 1.0)

            # persistent tail planes [*, BC] (ones rows written once)
            xqp = cst.tile([XQ, BC], fp16)
            _r, _c, _o = _BLOB["statt"]
            nc.gpsimd.dma_start(out=xqp[R3:R3 + 3, :], in_=blob[0:3, _o:_o + _c])
            nc.gpsimd.dma_start(out=xqp[XQ - 1:XQ, :], in_=onesr[:, 0:BC])
            hr1p = cst.tile([R1 + 1, BC], fp16)
            nc.gpsimd.dma_start(out=hr1p[R1:R1 + 1, :], in_=onesr[:, 0:BC])
            hr2p = cst.tile([R2 + 1, BC], fp16)
            nc.gpsimd.dma_start(out=hr2p[R2:R2 + 1, :], in_=onesr[:, 0:BC])
            hq1ap = cst.tile([128, BC], fp16)
            hq1bp = cst.tile([Q1 - 128 + 1, BC], fp16)
            nc.gpsimd.dma_start(out=hq1bp[Q1 - 128:Q1 - 128 + 1, :],
                                in_=onesr[:, 0:BC])
            hq2p = cst.tile([Q2 + 1, BC], fp16)
            nc.gpsimd.dma_start(out=hq2p[Q2:Q2 + 1, :], in_=onesr[:, 0:BC])
            e_sbp = cst.tile([3, BC], f32)
            o_all = cst.tile([QS, 12], f32)      # [sample, quarter*3+c] staging

            # warm-up matmuls: back-to-back, bridging into L1 (ramp the PE)
            pw = ps.tile([128, 512], f32, name="pw", tag="hp")
            for i in range(4):
                nc.tensor.matmul(pw[:, :], wtiny[:, 0:128], wtiny[:, :],
                                 start=(i == 0), stop=(i == 3))
            wsink = cst.tile([32, 2], fp16)
            nc.vector.tensor_scalar_max(wsink[:, :], pw[0:32, 0:2], 0.0)

            # ---- drain helper: whole-tile relu drain on ACT or DVE by balance
            def drain(dst, src, ncols):
                est_a = 0.85 * ncols + 220.0
                est_d = 1.04 * ncols + 190.0
                if bal["act"] + est_a <= bal["dve"] + est_d:
                    bal["act"] += est_a
                    nc.scalar.activation(dst, src, Relu)
                else:
                    bal["dve"] += est_d
                    nc.vector.tensor_scalar_max(dst, src, 0.0)

            # ---- phi layer sweeps ----
            def phi_l1(h, x_sb, slot=None, order=None):
                # L1 blocks of L1B cols round-robin over PE row-groups
                bpt = PT // L1B
                for t in (order if order is not None else range(TPH)):
                    if slot is not None:
                        slot(1)
                    p1 = ps.tile([128, PT], f32, name="p1", tag="hp")
                    for cc in range(bpt):
                        b = bpt * t + cc
                        j = b % 4
                        bl = b // 4
                        nc.tensor.matmul(
                            p1[:, cc * L1B:(cc + 1) * L1B],
                            blob_sb[32 * j:32 * j + 4, 0:128],
                            x_sb[32 * j:32 * j + 4, bl * L1B:(bl + 1) * L1B],
                            start=True, stop=True, tile_position=(32 * j, 0))
                    drain(h1s[h][:, t * PT:(t + 1) * PT], p1[:, :], PT)

            def phi_l2(h, slot=None):
                for t in range(TPH):
                    if slot is not None:
                        slot(1)
                    p2 = ps.tile([128, PT], f32, name="p2", tag="hp")
                    for cc in range(2):
                        c0 = t * PT + cc * 512
                        nc.tensor.matmul(p2[:, cc * 512:(cc + 1) * 512],
                                         w2s, h1s[h][:, c0:c0 + 512],
                                         start=True, stop=True)
                    drain(h2s[h][:, t * PT:(t + 1) * PT], p2[:, :], PT)

            def phi_l3_pair(h, p, slot=None):
                # tile A: m in [0,64); tile B: m in [64,128) of 16 samples.
                # ACT relus A whole-tile into scratch; one DVE STT folds
                # relu(B) + A into s (first pooling level fused into drain).
                pA = ps.tile([128, PT], f32, name="p3a", tag="hp")
                pB = ps.tile([128, PT], f32, name="p3b", tag="hp")
                for cc in range(2):
                    cA = (2 * p) * PT + cc * 512
                    nc.tensor.matmul(pA[:, cc * 512:(cc + 1) * 512],
                                     w3s, h2s[h][:, cA:cA + 512],
                                     start=True, stop=True)
                if slot is not None:
                    slot(1)
                for cc in range(2):
                    cB = (2 * p + 1) * PT + cc * 512
                    nc.tensor.matmul(pB[:, cc * 512:(cc + 1) * 512],
                                     w3s, h2s[h][:, cB:cB + 512],
                                     start=True, stop=True)
                h3ra = scr.tile([F3, PT], fp16, name="h3ra", tag="h3ra", bufs=2)
                nc.scalar.activation(h3ra[:, :], pA[0:F3, :], Relu)
                bal["act"] += 0.85 * PT + 220.0
                nc.vector.scalar_tensor_tensor(
                    s_hs[h][:, p * PT:(p + 1) * PT],
                    pB[0:F3, :], 0.0, h3ra[:, :],
                    op0=Alu.max, op1=Alu.add)
                bal["dve"] += 1.04 * PT + 190.0

            # ---- pooling trees (from 64 per sample) per 64-sample quarter;
            # whole tree on Pool (GpSimd) — it has no PSUM port so this is
            # the one big job it can absorb, freeing DVE for drains.
            def trees_q(q):
                h = q // 2
                s_sl = s_hs[h][:, (q % 2) * 4096:(q % 2) * 4096 + 4096]
                a3 = s_sl.rearrange("p (g m) -> p g m", m=64)
                t1 = scr.tile([F3, 2048], fp16, name=f"t1_{q}", tag="t1", bufs=1)
                o3 = t1[:, :].rearrange("p (g m) -> p g m", m=32)
                nc.gpsimd.tensor_tensor(out=o3, in0=a3[:, :, 0:32],
                                        in1=a3[:, :, 32:64], op=Alu.add)
                w = 32
                srt = t1[:, :]
                lv = 0
                while w > 1:
                    w //= 2
                    if w > 1:
                        nxt = scr.tile([F3, 64 * w], fp16, name=f"tl{lv}",
                                       tag=f"tl{lv}", bufs=2)[:, :]
                    else:
                        nxt = pooled[0:F3, q * QS:(q + 1) * QS]
                    a = srt.rearrange("p (g m) -> p g m", m=2 * w)
                    nc.gpsimd.tensor_tensor(
                        out=nxt.rearrange("p (g m) -> p g m", m=w) if w > 1 else nxt,
                        in0=a[:, :, 0:w], in1=a[:, :, w:2 * w], op=Alu.add)
                    srt = nxt
                    lv += 1

            # per-pair staggered tree for the last quarter: each L3 pair's 16
            # samples reduce right after their STT lands, so only the final
            # pair's short chain is exposed at the end.
            def tree_pair(h, p):
                q = 2 * h + p // 4
                s_sl = s_hs[h][:, p * PT:(p + 1) * PT]      # [80, 16*64]
                srt = s_sl
                w = 64
                lv = 0
                while w > 1:
                    w //= 2
                    if w > 1:
                        nxt = scr.tile([F3, 16 * w], fp16, name=f"pt{lv}",
                                       tag=f"pt{lv}", bufs=2)[:, :]
                    else:
                        nxt = pooled[0:F3, (p * 16) % BC + h * SAMP_HALF:
                                     (p * 16) % BC + h * SAMP_HALF + 16]
                    a = srt.rearrange("p (g m) -> p g m", m=2 * w)
                    nc.vector.tensor_tensor(
                        out=nxt.rearrange("p (g m) -> p g m", m=w) if w > 1 else nxt,
                        in0=a[:, :, 0:w], in1=a[:, :, w:2 * w], op=Alu.add)
                    srt = nxt
                    lv += 1
                bal["dve"] += 1500.0

            # ---- tail for one 64-sample quarter, emitted as a list of
            # closures so links can be interleaved between phi tiles.
            def tail_links(q):
                sl = slice(q * QS, (q + 1) * QS)
                links = []

                def l_r1():
                    pr1 = pst.tile([R1, QS], f32, name=f"pr1_{q}", tag="tail")
                    nc.tensor.matmul(pr1[:, :], r1s, pooled[:, sl],
                                     start=True, stop=True)
                    nc.scalar.activation(hr1p[0:R1, sl], pr1[:, :], Relu)

                def l_r2():
                    pr2 = pst.tile([R2, QS], f32, name=f"pr2_{q}", tag="tail")
                    nc.tensor.matmul(pr2[:, :], r2s, hr1p[:, sl],
                                     start=True, stop=True)
                    nc.scalar.activation(hr2p[0:R2, sl], pr2[:, :], Relu)

                def l_r3():
                    pr3 = pst.tile([R3, QS], f32, name=f"pr3_{q}", tag="tail")
                    nc.tensor.matmul(pr3[:, :], r3s, hr2p[:, sl],
                                     start=True, stop=True)
                    nc.scalar.copy(xqp[0:R3, sl], pr3[:, :])

                def l_q1():
                    pq1a = pst.tile([128, QS], f32, name=f"pq1a_{q}", tag="tail")
                    pq1b = pst.tile([Q1 - 128, QS], f32, name=f"pq1b_{q}", tag="tail")
                    nc.tensor.matmul(pq1a[:, :], q1as, xqp[:, sl],
                                     start=True, stop=True)
                    nc.tensor.matmul(pq1b[:, :], q1bs, xqp[:, sl],
                                     start=True, stop=True)
                    nc.scalar.activation(hq1ap[:, sl], pq1a[:, :], Relu)
                    nc.vector.tensor_scalar_max(hq1bp[0:Q1 - 128, sl],
                                                pq1b[:, :], 0.0)

                def l_q2():
                    pq2 = pst.tile([Q2, QS], f32, name=f"pq2_{q}", tag="tail")
                    nc.tensor.matmul(pq2[:, :], q2as, hq1ap[:, sl],
                                     start=True, stop=False)
                    nc.tensor.matmul(pq2[:, :], q2bs, hq1bp[:, sl],
                                     start=False, stop=True)
                    nc.scalar.activation(hq2p[0:Q2, sl], pq2[:, :], Relu)

                def l_q3():
                    pq3 = pst.tile([Q3, QS], f32, name=f"pq3_{q}", tag="tail")
                    nc.tensor.matmul(pq3[:, :], q3as, hq2p[:, sl],
                                     start=True, stop=True)
                    nc.scalar.activation(e_sbp[:, sl], pq3[:, :], Exp)

                rec = cst.tile([1, QS], f32, name=f"rec{q}")

                def l_sm1():
                    ssum = pst.tile([1, QS], f32, name=f"ssum{q}", tag="tail")
                    nc.tensor.matmul(ssum[:, :], ones3[:, :], e_sbp[:, sl],
                                     start=True, stop=True)
                    nc.vector.reciprocal(rec[:, :], ssum[:, :])

                def l_sm2():
                    eT = pst.tile([QS, 3], f32, name=f"eT{q}", tag="tail")
                    nc.tensor.transpose(eT[:, :], e_sbp[:, sl], eye3s[:, :])
                    rT = pst.tile([QS, 1], f32, name=f"rT{q}", tag="tail")
                    nc.tensor.transpose(rT[:, :], rec[:, :], eye1[:, :])
                    rTs = cst.tile([QS, 1], f32, name=f"rTs{q}")
                    nc.vector.tensor_copy(rTs[:, :], rT[:, :])
                    nc.vector.tensor_scalar_mul(o_all[:, 3 * q:3 * q + 3],
                                                eT[:, :], rTs[:, :])

                links += [l_r1, l_r2, l_r3, l_q1, l_q2, l_q3, l_sm1, l_sm2]
                bal["act"] += 1500.0
                bal["dve"] += 700.0
                return links

            # ---- emission schedule ----
            pending = []               # tail links awaiting interleave slots

            def emit_pending(n=1):
                for _ in range(n):
                    if pending:
                        pending.pop(0)()

            # even tiles (x groups 0,1 — first to arrive) before odd tiles
            phi_l1(0, x_sb0, order=[0, 2, 4, 6, 1, 3, 5, 8, 10, 7, 12, 9,
                                    14, 11, 13, 15])
            # x half 1 DMAs early on the idle gpsimd ring
            x_sb1 = xp.tile([128, HALF // 4], fp16, name="x_sb1", tag="x", bufs=2)
            for j in range(4):
                nc.gpsimd.dma_start(out=x_sb1[32 * j:32 * j + 4, :], in_=xin[1, j])
            phi_l2(0)
            for p in range(NPAIR):
                phi_l3_pair(0, p)
                if p == 3:
                    trees_q(0)
                    pending += tail_links(0)
                emit_pending(1)
            trees_q(1)
            pending += tail_links(1)
            # half 1: interleave remaining tail links 1-per-slot into the
            # L1/L2/L3 sweeps (a tail link between phi tiles hides its
            # matmul->drain round trip under the next tile's matmuls)
            phi_l1(1, x_sb1, slot=emit_pending)
            phi_l2(1, slot=emit_pending)
            for p in range(NPAIR):
                phi_l3_pair(1, p, slot=emit_pending)
                if p == 3:
                    trees_q(2)
                    pending += tail_links(2)
                if p >= 4:
                    tree_pair(1, p)      # stagger the last quarter's pooling
                emit_pending(1)
            # zip any leftover tail(2) links with tail(3): the two chains are
            # independent, so interleaving runs them concurrently
            q3links = tail_links(3)
            while pending or q3links:
                if pending:
                    pending.pop(0)()
                if q3links:
                    q3links.pop(0)()
            # single output DMA: one completion-latency chain instead of four
            nc.sync.dma_start(
                out=out.rearrange("(q s) c -> s q c", q=4),
                in_=o_all[:, :].rearrange("p (q c) -> p q c", q=4))

    nc.compile()
    return nc


def _prep_inputs(dyn, static, phi_w1, phi_b1, phi_w2, phi_b2, phi_w3, phi_b3,
                 rho_w1, rho_b1, rho_w2, rho_b2, rho_w3, rho_b3,
                 q_w1, q_b1, q_w2, q_b2, q_w3, q_b3):
    """Build per-core input maps (host-side layout transforms)."""
    fp16 = np.float16

    def aug_t(w, b):
        # [out, in] weight + bias -> transposed augmented [in+1, out]
        return np.concatenate([w, b[:, None]], axis=1).T.astype(fp16)

    q1 = aug_t(q_w1, q_b1)               # [44, 200]
    q2 = aug_t(q_w2, q_b2)               # [201, 100]
    # phi weights padded to [*, 128] / [128, 128] for FWL; bias row moves to
    # row 127 (fed by the previous layer's self-computed ones row), and
    # column 127 carries a unit weight from the previous ones row so the
    # ones row propagates (h1[127] = h2[127] = 1).
    w1a = np.zeros((4, 128), dtype=fp16)
    w1a[:, 0:F1] = aug_t(phi_w1, phi_b1)
    w1a[3, 127] = 1.0                    # x ones row -> h1[127] = 1
    w2a = np.zeros((128, 128), dtype=fp16)
    w2a[0:F1, 0:F2] = phi_w2.T.astype(fp16)
    w2a[127, 0:F2] = phi_b2.astype(fp16)
    w2a[127, 127] = 1.0                  # h1 ones row -> h2[127] = 1
    w3a = np.zeros((128, 128), dtype=fp16)
    w3a[0:F2, 0:F3] = phi_w3.T.astype(fp16)
    w3a[127, 0:F3] = phi_b3.astype(fp16)
    parts = dict(
        w1a=w1a, w2a=w2a, w3a=w3a, r1a=aug_t(rho_w1, rho_b1),
        r2a=aug_t(rho_w2, rho_b2), r3a=aug_t(rho_w3, rho_b3),
        q1aw=q1[:, 0:128], q1bw=q1[:, 128:],
        q2aw=q2[0:128, :], q2bw=q2[128:, :], q3aw=aug_t(q_w3, q_b3))

    eye3 = np.eye(3, dtype=np.float32)
    onesr = np.ones((1, HALF), dtype=fp16)

    base_blob = np.zeros((128, BLOBW), dtype=fp16)
    for name, (r, cc, o) in _BLOB.items():
        if name != "statt":
            base_blob[0:r, o:o + cc] = parts[name]
    for j in range(1, 4):   # replicate L1 weights into each PE row-group
        base_blob[32 * j:32 * j + 4, 0:128] = parts["w1a"]

    # element order within a half: tile T (1024 cols), pair p = T//2:
    # tile A (T even): col c -> sample p*16 + c//64, m = c%64
    # tile B (T odd):  col c -> sample p*16 + c//64, m = 64 + c%64
    # Build permutation: halfcol g -> (sample_in_half, m)
    Tl = np.arange(HALF) // PT
    cl = np.arange(HALF) % PT
    pl = Tl // 2
    samp = pl * 16 + cl // 64
    mm = (Tl % 2) * 64 + cl % 64
    elem_of_col = samp * M + mm          # index into half's [2048*M?] no: per-half

    in_maps = []
    for c in range(N_CORES):
        blob = base_blob.copy()
        r, cc, o = _BLOB["statt"]
        blob[0:r, o:o + cc] = static[c * BC:(c + 1) * BC].T.astype(fp16)
        xc = dyn[c * BC:(c + 1) * BC].reshape(EC, D)
        # xin[h, j, row, t*256 + n] = comp(row) of element at tile t, block j
        xin = np.empty((2, 4, 4, HALF // 4), dtype=fp16)
        for hh in range(2):
            xh = xc[hh * HALF:(hh + 1) * HALF]          # [16384, 3]
            xperm = xh[elem_of_col]                      # cols in emission order
            comp = np.concatenate([xperm, np.ones((HALF, 1), np.float32)],
                                  axis=1)                # [HALF, 4] comps
            # blocks of L1B cols RR over groups: block b -> group b%4,
            # group-local slot b//4
            blocks = comp.reshape(HALF // L1B, L1B, 4)
            for j in range(4):
                grp = blocks[j::4]                   # [HALF//L1B//4, L1B, 4]
                xin[hh, j] = grp.transpose(2, 0, 1).reshape(4, HALF // 4)
        in_maps.append(dict(xin=xin, blob=blob, onesr=onesr, eye3=eye3))
    return in_maps


def kernel(**inputs):
    import time
    from concourse.bass_utils import run_bass_kernel_spmd

    if "nc" not in _compiled:
        _compiled["nc"] = _build()
    nc = _compiled["nc"]

    in_maps = _prep_inputs(**inputs)
    last_err = None
    for attempt in range(3):
        try:
            res = run_bass_kernel_spmd(nc, in_maps, core_ids=list(range(N_CORES)))
            break
        except Exception as e:          # transient device errors: back off and retry
            last_err = e
            time.sleep(20 * (attempt + 1))
    else:
        raise last_err
    out = np.concatenate([res.results[c]["out"] for c in range(N_CORES)], axis=0)
    return out.astype(np.float32)
